# revision 43
# baseline (speedup 1.0000x reference)
"""Sparse (sliding-window) GQA attention prefill kernel for 8 Trainium2 cores.

Sharding: TP=4 over KV heads (2 KV heads + 10 Q heads per core) x DP=2 over
batch. Core c: batch = c // 4, shard q4 = c % 4.

Device program (SPMD, identical on all cores; per-core data via inputs):
  A1: xqT/xkT projections in transposed layout ([head_dim, seq]); sum-of-squares
      for the global RMS norm via Square + ones-matmul; per-s-tile AllReduce of
      the norm partials within each batch group (overlaps A1 compute).
  A2: V projection in natural layout ([seq, head_dim]).
  R:  norm rows -> rope tables (bf16, norm factor folded in); batched rope over
      all local heads per 512-slice (vector for q, gpsimd for k), overlapping A2.
  B:  per (head-pair chunk, 2-query-tile group): scoresT = K^T-chunk.T @ qT in
      the sliding band, mask add, exp, ones-matmul denominator, P^T @ V
      accumulation. Software-pipelined: the denominator/PV matmuls for step j
      are emitted after the scores matmul of step j+1 so the PE never waits on
      the exp. Divide on evacuation via broadcast + reciprocal on [128, .].
      attnT shipped in s-halves: AllGather per (chunk, half), issued late so the
      collective's input wait never blocks the gpsimd queue.
  C:  out = attnT.T @ wo col-shard; first s-half rows start as soon as the
      half-0 gathers land.
"""

import sys
import numpy as np

for _p in ("/opt/trn_rl_repo", "/root/.axon_site/_ro/trn_rl_repo"):
    if _p not in sys.path:
        sys.path.insert(0, _p)

import ml_dtypes

import concourse.bass as bass
import concourse.tile as tile
from concourse import bacc, mybir
from concourse import bass_utils

F32 = mybir.dt.float32
BF16 = mybir.dt.bfloat16
BF16_NP = ml_dtypes.bfloat16
AF = mybir.ActivationFunctionType
ALU = mybir.AluOpType


class Cfg:
    def __init__(self, S=2048, DIM=5120, HQ=40, HKV=8, TP=4, DP=2, SW=1024,
                 MSCALE=1.2079441541679836, EPS=1e-6):
        self.S, self.DIM, self.HQ, self.HKV = S, DIM, HQ, HKV
        self.TP, self.DP, self.SW = TP, DP, SW
        self.MSCALE, self.EPS = MSCALE, EPS
        self.D = 128
        self.NC = TP * DP
        self.HQL = HQ // TP          # local q heads
        self.KVL = HKV // TP         # local kv heads
        self.REP = HQ // HKV
        self.KC = DIM // 128         # contraction chunks
        self.NT = S // 128           # seq tiles
        self.G = self.NT // 2        # 2-query-tile groups
        self.WD = SW // 128          # window in tiles
        self.COLS = DIM // TP        # output column shard
        self.HCL = self.HQL + self.KVL  # projection chains with transposed out
        self.NST = S // 512          # 512-wide s-tiles (phase A1)
        self.NST2 = S // 256         # 256-wide s-tiles (phase A2)
        self.CQ = self.D ** -0.5 * MSCALE
        assert self.WD >= 2 and self.NT > self.WD + 1 and self.NT % 2 == 0
        self.groups = [[b * TP + r for r in range(TP)] for b in range(DP)]


def head_chunks(C):
    """Per-kv head pair chunks: [(kv, [h0,h1]), (kv, [h2,h3]), (kv, [h4])...]"""
    out = []
    per = C.HQL // C.KVL
    for kv in range(C.KVL):
        hs = list(range(kv * per, (kv + 1) * per))
        i = 0
        while i < len(hs):
            out.append((kv, hs[i:i + 2]))
            i += 2
    return out


def attention_tile_kernel(tc, C, io):
    nc = tc.nc
    S, KC, HQL, KVL, NT, G, WD = C.S, C.KC, C.HQL, C.KVL, C.NT, C.G, C.WD
    H2 = S // 2
    xT16, wqkv, wv_in, wo_in = io["xT16"], io["wqkv"], io["wv_in"], io["wo_in"]
    cosT_in, sinT_in, masks_in, wnorm_in = io["cosT"], io["sinT"], io["masks"], io["wnorm"]
    out_sh = io["out_sh"]
    chunks = head_chunks(C)

    from contextlib import ExitStack
    ctx = ExitStack()
    with ctx:
        singles = ctx.enter_context(tc.tile_pool(name="singles", bufs=1))
        dramcc = ctx.enter_context(tc.tile_pool(name="dramcc", bufs=1, space="DRAM"))

        ones16 = singles.tile([128, 1], BF16)
        nc.vector.memset(ones16[:], 1.0)
        ones128 = singles.tile([128, 128], BF16)
        nc.vector.memset(ones128[:], 1.0)
        wnorm_sb = singles.tile([128, HQL + KVL], F32)
        nc.sync.dma_start(wnorm_sb[:], wnorm_in[:])
        cq2 = C.CQ * C.CQ
        eps_q = singles.tile([128, 1], F32)
        nc.vector.memset(eps_q[:], C.EPS / cq2)
        eps_k = singles.tile([128, 1], F32)
        nc.vector.memset(eps_k[:], C.EPS)

        cc_nins = [dramcc.tile([1, 1024], F32, name=f"ccni{st}")
                   for st in range(C.NST)]
        cc_nouts = [dramcc.tile([1, 1024], F32, name=f"ccno{st}")
                    for st in range(C.NST)]
        # one gather per 512-wide s-range (phase B runs g-outer), so phase C
        # consumes s-slabs progressively and never waits on the last
        # collective; 4 gathers keeps the ~40us fixed CC cost per collective
        # well under phase B's span
        NGA = G // 2
        cc_ains = [dramcc.tile([HQL, 128, 512], BF16, name=f"ccag{gi}")
                   for gi in range(NGA)]
        cc_aouts = [dramcc.tile([C.TP, HQL, 128, 512], BF16,
                                name=f"ccaog{gi}") for gi in range(NGA)]

        wvp = ctx.enter_context(tc.tile_pool(name="wvp", bufs=1))
        wv_sb = wvp.tile([128, KC, KVL * 128], BF16)
        nc.gpsimd.dma_start(wv_sb[:], wv_in[:])

        with (
            tc.tile_pool(name="xqp", bufs=1) as xq_pool,
            tc.tile_pool(name="xkp", bufs=1) as xk_pool,
            tc.tile_pool(name="vp", bufs=1) as v_pool,
        ):
            xq_sb = xq_pool.tile([128, HQL, S], BF16)
            xk_sb = xk_pool.tile([128, KVL, S], BF16)
            v_sb = v_pool.tile([128, NT, KVL, 128], BF16)

            # ---- phase A: q/k projections (transposed out) + norm
            # partials + V projection, all per 512-wide s-tile. V reuses the
            # same xt tiles as stationary, so x is loaded exactly once.
            # Rope chains are emitted with a 2-tile lag so each slice's
            # AllReduce has completed long before its chain runs; slices 0-1
            # rope on the vector engine during A itself.
            KCH = KC // 2
            with (
                tc.tile_pool(name="xt1", bufs=2) as xt1,
                tc.tile_pool(name="wst", bufs=2) as wst,
                tc.tile_pool(name="sqp", bufs=2) as sqp,
                tc.tile_pool(name="trow", bufs=1) as trow,
                tc.tile_pool(name="tabsQ", bufs=1) as tabs_q,
                tc.tile_pool(name="rowsp", bufs=1) as rowsp,
                tc.tile_pool(name="ropep", bufs=1) as ropep,
                tc.tile_pool(name="psA", bufs=3, space="PSUM") as psA,
                tc.tile_pool(name="psN", bufs=1, space="PSUM") as psN,
                tc.tile_pool(name="psV", bufs=3, space="PSUM") as psV,
            ):

                def emit_rchain(st):
                    sl = slice(st * 512, (st + 1) * 512)
                    # broadcast raw partial sums, then r = exp(-0.5*ln(ax+b))
                    rowraw = rowsp.tile([1, 1024], F32, tag="rowraw")
                    nc.gpsimd.dma_start(rowraw[:], cc_nouts[st][:])
                    rq_b = rowsp.tile([128, 512], F32, tag="rqb")
                    rk_b = rowsp.tile([128, 512], F32, tag="rkb")
                    nc.gpsimd.partition_broadcast(rk_b[:],
                                                  rowraw[0:1, 512:1024])
                    nc.gpsimd.partition_broadcast(rq_b[:],
                                                  rowraw[0:1, 0:512])
                    nc.scalar.activation(rk_b[:], rk_b[:], AF.Ln,
                                         scale=1.0 / (C.HKV * 128),
                                         bias=eps_k[:])
                    nc.scalar.activation(rk_b[:], rk_b[:], AF.Exp, scale=-0.5)
                    nc.scalar.activation(rq_b[:], rq_b[:], AF.Ln,
                                         scale=1.0 / (C.DIM * cq2),
                                         bias=eps_q[:])
                    nc.scalar.activation(rq_b[:], rq_b[:], AF.Exp, scale=-0.5)
                    cosq = tabs_q.tile([128, 512], BF16, tag="cosq")
                    sinq = tabs_q.tile([128, 512], BF16, tag="sinq")
                    cosk = tabs_q.tile([128, 512], BF16, tag="cosk")
                    sink = tabs_q.tile([128, 512], BF16, tag="sink")
                    nc.gpsimd.dma_start(cosk[:], cosT_in[:, sl])
                    nc.gpsimd.dma_start(sink[:], sinT_in[:, sl])
                    nc.gpsimd.dma_start(cosq[:], cosT_in[:, sl])
                    nc.gpsimd.dma_start(sinq[:], sinT_in[:, sl])
                    nc.vector.tensor_mul(cosk[:], cosk[:], rk_b[:])
                    nc.vector.tensor_mul(sink[:], sink[:], rk_b[:])
                    nc.vector.tensor_mul(cosq[:], cosq[:], rq_b[:])
                    nc.vector.tensor_mul(sinq[:], sinq[:], rq_b[:])
                    rotk = ropep.tile([128, KVL, 512], BF16, tag="rotk")
                    rotq = ropep.tile([128, HQL, 512], BF16, tag="rotq")
                    nc.gpsimd.dma_start(rotk[0:64], xk_sb[64:128, :, sl])
                    nc.gpsimd.dma_start(rotk[64:128], xk_sb[0:64, :, sl])
                    nc.gpsimd.dma_start(rotq[0:64], xq_sb[64:128, :, sl])
                    nc.gpsimd.dma_start(rotq[64:128], xq_sb[0:64, :, sl])
                    # fully in-place on vector: rot *= sin; x *= cos; x += rot
                    nc.vector.tensor_mul(
                        rotk[:], rotk[:],
                        sink[:, None, :].to_broadcast((128, KVL, 512)))
                    nc.vector.tensor_mul(
                        xk_sb[:, :, sl], xk_sb[:, :, sl],
                        cosk[:, None, :].to_broadcast((128, KVL, 512)))
                    nc.vector.tensor_add(xk_sb[:, :, sl], xk_sb[:, :, sl],
                                         rotk[:])
                    nc.vector.tensor_mul(
                        rotq[:], rotq[:],
                        sinq[:, None, :].to_broadcast((128, HQL, 512)))
                    nc.vector.tensor_mul(
                        xq_sb[:, :, sl], xq_sb[:, :, sl],
                        cosq[:, None, :].to_broadcast((128, HQL, 512)))
                    nc.vector.tensor_add(xq_sb[:, :, sl], xq_sb[:, :, sl],
                                         rotq[:])

                for st in range(C.NST):
                    if st >= 2:
                        emit_rchain(st - 2)
                    s0 = st * 512
                    xt_a = xt1.tile([128, KCH, 512], BF16, tag="xta")
                    xt_b = xt1.tile([128, KCH, 512], BF16, tag="xtb")
                    nc.sync.dma_start(
                        xt_a[:],
                        xT16[:KCH, :, s0:s0 + 512].rearrange("kc p s -> p kc s"))
                    nc.sync.dma_start(
                        xt_b[:],
                        xT16[KCH:, :, s0:s0 + 512].rearrange("kc p s -> p kc s"))
                    ps_nq = psN.tile([1, 512], F32, tag="nq")
                    ps_nk = psN.tile([1, 512], F32, tag="nk")
                    for hc in range(C.HCL):
                        w_sb = wst.tile([128, KC, 128], BF16, tag="w")
                        nc.sync.dma_start(w_sb[:], wqkv[hc])
                        ps = psA.tile([128, 512], F32, tag="proj")
                        for kc in range(KC):
                            xsrc = xt_a if kc < KCH else xt_b
                            nc.tensor.matmul(ps[:], w_sb[:, kc, :],
                                             xsrc[:, kc % KCH, :],
                                             start=(kc == 0), stop=(kc == KC - 1))
                        if hc < HQL:
                            dest = xq_sb[:, hc, s0:s0 + 512]
                        else:
                            dest = xk_sb[:, hc - HQL, s0:s0 + 512]
                        nc.vector.tensor_scalar_mul(dest, ps[:],
                                                    wnorm_sb[:, hc:hc + 1])
                        sq = sqp.tile([128, 512], BF16, tag="sq")
                        nc.scalar.activation(sq[:], ps[:], AF.Square)
                        tgt = ps_nq if hc < HQL else ps_nk
                        first = (hc == 0) or (hc == HQL)
                        last = (hc == HQL - 1) or (hc == C.HCL - 1)
                        nc.tensor.matmul(tgt[:], ones16[:], sq[:],
                                         start=first, stop=last)
                    rq_t = trow.tile([1, 512], F32, tag="rq")
                    rk_t = trow.tile([1, 512], F32, tag="rk")
                    nc.vector.tensor_copy(rq_t[:], ps_nq[:])
                    nc.vector.tensor_copy(rk_t[:], ps_nk[:])
                    nc.sync.dma_start(cc_nins[st][0:1, 0:512], rq_t[:])
                    nc.sync.dma_start(cc_nins[st][0:1, 512:1024], rk_t[:])
                    # per-s-tile AllReduce of norm partials: overlaps A compute
                    nc.gpsimd.collective_compute(
                        "AllReduce", ALU.add, replica_groups=C.groups,
                        ins=[cc_nins[st].opt()], outs=[cc_nouts[st].opt()])
                    # V projection for this s-tile, x chunks as stationary
                    for tc4 in range(4):
                        tt = st * 4 + tc4
                        psv = psV.tile([128, KVL * 128], F32, tag="v")
                        for kc in range(KC):
                            xsrc = xt_a if kc < KCH else xt_b
                            nc.tensor.matmul(
                                psv[:],
                                xsrc[:, kc % KCH,
                                     tc4 * 128:(tc4 + 1) * 128],
                                wv_sb[:, kc, :],
                                start=(kc == 0), stop=(kc == KC - 1))
                        nc.scalar.copy(v_sb[:, tt, :, :], psv[:])

                emit_rchain(C.NST - 2)
                emit_rchain(C.NST - 1)

            # ---- phase B: banded attention, software-pipelined --------------
            with (
                tc.tile_pool(name="attnp", bufs=1) as attnp,
                tc.tile_pool(name="maskp", bufs=1) as maskp,
                tc.tile_pool(name="expp", bufs=4) as expp,
                tc.tile_pool(name="bmisc", bufs=3) as bmisc,
                tc.tile_pool(name="psSC", bufs=2, space="PSUM") as psSC,
                tc.tile_pool(name="psAT", bufs=2, space="PSUM") as psAT,
                tc.tile_pool(name="psDN", bufs=2, space="PSUM") as psDN,
            ):
                attnT = attnp.tile([128, HQL, S], BF16)
                masks_sb = maskp.tile([128, 4, 256], F32)
                nc.sync.dma_start(masks_sb[:], masks_in[:])
                off2m = {0: 0, 1: 1, WD: 2, WD + 1: 3}

                for g in range(G):
                    jlo, jhi = max(0, 2 * g - WD), 2 * g + 1
                    npairs = (jhi - jlo + 1) // 2
                    for ci, (kv, hs) in enumerate(chunks):
                        w = len(hs)
                        h0 = hs[0]
                        ps_at = psAT.tile([128, 512], F32, tag="at")
                        # ones128 stationary -> denominator lands pre-broadcast
                        # across all 128 partitions (and avoids the 1-wide
                        # stationary pipeline penalty)
                        ps_dn = psDN.tile([128, 512], F32, tag="dn")
                        pend = []  # (ex slice, j) waiting for dn/at emission

                        def drain():
                            for exp_, jp in pend:
                                nc.tensor.matmul(ps_dn[:, :w * 256],
                                                 ones128[:],
                                                 exp_, start=(jp == jlo),
                                                 stop=(jp == jhi))
                                nc.tensor.matmul(ps_at[:, :w * 256],
                                                 v_sb[:, jp, kv, :],
                                                 exp_, start=(jp == jlo),
                                                 stop=(jp == jhi))
                            pend.clear()

                        for p in range(npairs):
                            j0 = jlo + 2 * p
                            ps2 = psSC.tile([128, 1024], F32, tag="sc")
                            for dj in range(2):
                                j = j0 + dj
                                o = dj * 512
                                nc.tensor.matmul(
                                    ps2[:, o:o + w * 256].rearrange(
                                        "p (w s) -> p w s", w=w),
                                    xk_sb[:, kv, j * 128:(j + 1) * 128],
                                    xq_sb[:, h0:h0 + w, g * 256:(g + 1) * 256],
                                    start=True, stop=True)
                            # consume the previous pair while this pair's exp
                            # runs -> the PE never waits on the scalar engine
                            drain()
                            for dj in range(2):
                                j = j0 + dj
                                m = off2m.get(jhi - j)
                                if m is not None:
                                    o = dj * 512
                                    nc.vector.tensor_add(
                                        ps2[:, o:o + w * 256].rearrange(
                                            "p (w s) -> p w s", w=w),
                                        ps2[:, o:o + w * 256].rearrange(
                                            "p (w s) -> p w s", w=w),
                                        masks_sb[:, m, None, :].to_broadcast(
                                            (128, w, 256)))
                            ex2 = expp.tile([128, 1024], BF16, tag="ex")
                            nc.scalar.activation(
                                ex2.rearrange("p (j s) -> p j s",
                                              j=2)[:, :, :w * 256],
                                ps2.rearrange("p (j s) -> p j s",
                                              j=2)[:, :, :w * 256],
                                AF.Exp)
                            pend.append((ex2[:, 0:w * 256], j0))
                            pend.append((ex2[:, 512:512 + w * 256], j0 + 1))
                        drain()
                        den_b = bmisc.tile([128, 512], F32, tag="denb")
                        nc.vector.reciprocal_approx_fast(
                            out=den_b[:, :w * 256], in_=ps_dn[:, :w * 256])
                        nc.vector.tensor_mul(
                            attnT[:, h0:h0 + w, g * 256:(g + 1) * 256],
                            ps_at[:, :w * 256].rearrange(
                                "p (w s) -> p w s", w=w),
                            den_b[:, :w * 256].rearrange(
                                "p (w s) -> p w s", w=w))
                    # all heads for this s-range done: ship + gather the slab
                    # every second g. gpsimd is otherwise idle in B, so the
                    # collective's short input wait can't block anything.
                    if g % 2 == 1:
                        gi = g // 2
                        nc.sync.dma_start(
                            cc_ains[gi].rearrange("h p s -> p h s"),
                            attnT[:, :, (g - 1) * 256:(g + 1) * 256])
                        nc.gpsimd.collective_compute(
                            "AllGather", ALU.bypass,
                            replica_groups=C.groups,
                            ins=[cc_ains[gi].opt()], outs=[cc_aouts[gi].opt()])

        # ---- phase C: output projection, wo streamed in column panels ----
        # (a monolithic 13MB wo load would stall the PE for ~50us at B->C;
        # panel-outer order hides all but the first, smallest panel's load)
        with (
            tc.tile_pool(name="wop", bufs=2) as wop,
            tc.tile_pool(name="lhsp", bufs=2) as lhsp,
            tc.tile_pool(name="outp", bufs=3) as outp,
            tc.tile_pool(name="psO", bufs=3, space="PSUM") as psO,
        ):
            col_ts = [(0, 256), (256, 512), (768, 512)]
            for (c0, wdt) in col_ts:
                wo_p = wop.tile([128, C.HQ, 512], BF16, tag="wop")
                nc.sync.dma_start(wo_p[:, :, :wdt], wo_in[:, :, c0:c0 + wdt])
                for sb in range(NT):
                    gi, off = sb // 4, (sb % 4) * 128
                    lhs = lhsp.tile([128, C.HQ, 128], BF16, tag="lhs")
                    nc.sync.dma_start(
                        lhs[:],
                        cc_aouts[gi][:, :, :, off:off + 128].rearrange(
                            "r h p s -> p (r h) s"))
                    ps_o = psO.tile([128, 512], F32, tag="o")
                    for slot in range(C.HQ):
                        nc.tensor.matmul(ps_o[:, :wdt], lhs[:, slot, :],
                                         wo_p[:, slot, :wdt],
                                         start=(slot == 0),
                                         stop=(slot == C.HQ - 1))
                    oro = outp.tile([128, 512], F32, tag="oro")
                    nc.vector.tensor_copy(oro[:, :wdt], ps_o[:, :wdt])
                    nc.sync.dma_start(
                        out_sh[sb * 128:(sb + 1) * 128, c0:c0 + wdt],
                        oro[:, :wdt])


def build_program(C):
    nc = bacc.Bacc("TRN2", target_bir_lowering=False, debug=False,
                   num_devices=C.NC)
    io = {
        "xT16": nc.dram_tensor("xT16", [C.KC, 128, C.S], BF16, kind="ExternalInput").ap(),
        "wqkv": nc.dram_tensor("wqkv", [C.HCL, 128, C.KC, 128], BF16,
                               kind="ExternalInput").ap(),
        "wv_in": nc.dram_tensor("wv_in", [128, C.KC, C.KVL * 128], BF16,
                                kind="ExternalInput").ap(),
        "wo_in": nc.dram_tensor("wo_in", [128, C.HQ, C.COLS], BF16,
                                kind="ExternalInput").ap(),
        "cosT": nc.dram_tensor("cosT", [128, C.S], BF16, kind="ExternalInput").ap(),
        "sinT": nc.dram_tensor("sinT", [128, C.S], BF16, kind="ExternalInput").ap(),
        "masks": nc.dram_tensor("masks", [128, 4, 256], F32, kind="ExternalInput").ap(),
        "wnorm": nc.dram_tensor("wnorm", [128, C.HQL + C.KVL], F32,
                                kind="ExternalInput").ap(),
        "out_sh": nc.dram_tensor("out_sh", [C.S, C.COLS], F32,
                                 kind="ExternalOutput").ap(),
    }
    with tile.TileContext(nc) as tc:
        attention_tile_kernel(tc, C, io)
    nc.compile()
    return nc


def make_masks(mask_np, C):
    """4 mask tiles [t,s-pair] for offsets {0,1,WD,WD+1}; returns [128,4,256] f32."""
    S, WD, SW = C.S, C.WD, C.SW
    I0 = WD + 1

    def tileT(d):
        i, j = I0, I0 - d
        if 0 <= j < C.NT:
            blk = np.array(mask_np[i * 128:(i + 1) * 128, j * 128:(j + 1) * 128],
                           dtype=np.float64)
        else:
            blk = np.full((128, 128), -np.inf)
        s_idx = np.arange(128)[:, None]
        t_idx = np.arange(128)[None, :]
        dist = 128 * d + s_idx - t_idx
        blk = np.where(dist > SW, -np.inf, blk)
        return np.maximum(blk.T, -1e30).astype(np.float32)   # [t, s]

    tiles = []
    for off in (0, 1, WD, WD + 1):
        dl, dr = off - 1, off
        tiles.append(np.concatenate([tileT(dl), tileT(dr)], axis=1))
    return np.ascontiguousarray(np.stack(tiles, axis=1))      # [128, 4, 256]


def make_core_inputs(inputs, C):
    x = np.asarray(inputs["x"], dtype=np.float32)
    wq = np.asarray(inputs["wq"], dtype=np.float32)
    wk = np.asarray(inputs["wk"], dtype=np.float32)
    wv = np.asarray(inputs["wv"], dtype=np.float32)
    wo = np.asarray(inputs["wo"], dtype=np.float32)
    qw = np.asarray(inputs["q_norm_weight"], dtype=np.float32)
    kw = np.asarray(inputs["k_norm_weight"], dtype=np.float32)
    ch = np.asarray(inputs["cos_half"], dtype=np.float32)
    sh = np.asarray(inputs["sin_half"], dtype=np.float32)
    mask = np.asarray(inputs["mask"], dtype=np.float32)
    assert int(inputs.get("start_pos", 0) or 0) == 0

    cosT = np.ascontiguousarray(
        np.concatenate([ch.T, ch.T], axis=0)).astype(BF16_NP)
    sinT = np.ascontiguousarray(
        np.concatenate([-sh.T, sh.T], axis=0)).astype(BF16_NP)
    masks = make_masks(mask, C)
    KC, HQL, KVL = C.KC, C.HQL, C.KVL

    xT_cache = {}
    for b in range(C.DP):
        xT_cache[b] = np.ascontiguousarray(x[b].T).astype(BF16_NP).reshape(
            C.KC, 128, C.S)
    in_maps = []
    for c in range(C.NC):
        b, q4 = c // C.TP, c % C.TP
        x16 = xT_cache[b]
        wq_s = wq[:, 128 * HQL * q4:128 * HQL * (q4 + 1)]
        wk_s = wk[:, 128 * KVL * q4:128 * KVL * (q4 + 1)]
        wv_s = wv[:, 128 * KVL * q4:128 * KVL * (q4 + 1)]
        wqk = np.concatenate([wq_s, wk_s], axis=1).astype(BF16_NP)
        # [HCL, 128, KC, 128]: per chain, contraction-partition-major
        wqkv_pre = np.ascontiguousarray(
            wqk.reshape(KC, 128, C.HCL, 128).transpose(2, 1, 0, 3))
        wv_pre = np.ascontiguousarray(
            wv_s.astype(BF16_NP).reshape(KC, 128, KVL * 128).transpose(1, 0, 2))
        wo_s = wo[:, C.COLS * q4:C.COLS * (q4 + 1)].astype(BF16_NP)
        wo_pre = np.ascontiguousarray(
            wo_s.reshape(C.HQ, 128, C.COLS).transpose(1, 0, 2))
        wn = np.zeros((128, HQL + KVL), dtype=np.float32)
        for hc in range(HQL):
            g = HQL * q4 + hc
            wn[:, hc] = qw[128 * g:128 * (g + 1)]
        for j in range(KVL):
            g = KVL * q4 + j
            wn[:, HQL + j] = kw[128 * g:128 * (g + 1)]
        in_maps.append({"xT16": x16, "wqkv": wqkv_pre, "wv_in": wv_pre,
                        "wo_in": wo_pre, "cosT": cosT, "sinT": sinT,
                        "masks": masks, "wnorm": wn})
    return in_maps


_CACHED = {}


def run(inputs, C=None, trace=False, stitch=None, trace_cores=None):
    C = C or Cfg()
    key = (C.S, C.DIM, C.HQ, C.HKV, C.TP, C.DP, C.SW)
    if key not in _CACHED:
        _CACHED[key] = build_program(C)
    nc = _CACHED[key]
    in_maps = make_core_inputs(inputs, C)
    if stitch is None:
        stitch = trace
    if trace and trace_cores is None:
        trace_cores = list(range(C.NC))
    res = bass_utils.run_bass_kernel_spmd(
        nc, in_maps, core_ids=list(range(C.NC)), trace=trace,
        stitch_traces=stitch, trace_cores=trace_cores if trace else None)
    out = np.empty((C.DP, C.S, C.DIM), dtype=np.float32)
    for c in range(C.NC):
        b, q4 = c // C.TP, c % C.TP
        out[b, :, C.COLS * q4:C.COLS * (q4 + 1)] = res.results[c]["out_sh"]
    return out, res


def kernel(**inputs) -> np.ndarray:
    out, _ = run(inputs)
    return out


# revision 44
# speedup vs baseline: 1.0887x; 1.0887x over previous
"""Sparse (sliding-window) GQA attention prefill kernel for 8 Trainium2 cores.

Sharding: TP=4 over KV heads (2 KV heads + 10 Q heads per core) x DP=2 over
batch. Core c: batch = c // 4, shard q4 = c % 4.

Device program (SPMD, identical on all cores; per-core data via inputs):
  A1: xqT/xkT projections in transposed layout ([head_dim, seq]); sum-of-squares
      for the global RMS norm via Square + ones-matmul; per-s-tile AllReduce of
      the norm partials within each batch group (overlaps A1 compute).
  A2: V projection in natural layout ([seq, head_dim]).
  R:  norm rows -> rope tables (bf16, norm factor folded in); batched rope over
      all local heads per 512-slice (vector for q, gpsimd for k), overlapping A2.
  B:  per (head-pair chunk, 2-query-tile group): scoresT = K^T-chunk.T @ qT in
      the sliding band, mask add, exp, ones-matmul denominator, P^T @ V
      accumulation. Software-pipelined: the denominator/PV matmuls for step j
      are emitted after the scores matmul of step j+1 so the PE never waits on
      the exp. Divide on evacuation via broadcast + reciprocal on [128, .].
      attnT shipped in s-halves: AllGather per (chunk, half), issued late so the
      collective's input wait never blocks the gpsimd queue.
  C:  out = attnT.T @ wo col-shard; first s-half rows start as soon as the
      half-0 gathers land.
"""

import sys
import numpy as np

for _p in ("/opt/trn_rl_repo", "/root/.axon_site/_ro/trn_rl_repo"):
    if _p not in sys.path:
        sys.path.insert(0, _p)

import ml_dtypes

import concourse.bass as bass
import concourse.tile as tile
from concourse import bacc, mybir
from concourse import bass_utils

F32 = mybir.dt.float32
BF16 = mybir.dt.bfloat16
BF16_NP = ml_dtypes.bfloat16
AF = mybir.ActivationFunctionType
ALU = mybir.AluOpType


class Cfg:
    def __init__(self, S=2048, DIM=5120, HQ=40, HKV=8, TP=4, DP=2, SW=1024,
                 MSCALE=1.2079441541679836, EPS=1e-6):
        self.S, self.DIM, self.HQ, self.HKV = S, DIM, HQ, HKV
        self.TP, self.DP, self.SW = TP, DP, SW
        self.MSCALE, self.EPS = MSCALE, EPS
        self.D = 128
        self.NC = TP * DP
        self.HQL = HQ // TP          # local q heads
        self.KVL = HKV // TP         # local kv heads
        self.REP = HQ // HKV
        self.KC = DIM // 128         # contraction chunks
        self.NT = S // 128           # seq tiles
        self.G = self.NT // 2        # 2-query-tile groups
        self.WD = SW // 128          # window in tiles
        self.COLS = DIM // TP        # output column shard
        self.HCL = self.HQL + self.KVL  # projection chains with transposed out
        self.NST = S // 512          # 512-wide s-tiles (phase A1)
        self.NST2 = S // 256         # 256-wide s-tiles (phase A2)
        self.CQ = self.D ** -0.5 * MSCALE
        assert self.WD >= 2 and self.NT > self.WD + 1 and self.NT % 2 == 0
        self.groups = [[b * TP + r for r in range(TP)] for b in range(DP)]


def head_chunks(C):
    """Per-kv head pair chunks: [(kv, [h0,h1]), (kv, [h2,h3]), (kv, [h4])...]"""
    out = []
    per = C.HQL // C.KVL
    for kv in range(C.KVL):
        hs = list(range(kv * per, (kv + 1) * per))
        i = 0
        while i < len(hs):
            out.append((kv, hs[i:i + 2]))
            i += 2
    return out


def attention_tile_kernel(tc, C, io):
    nc = tc.nc
    S, KC, HQL, KVL, NT, G, WD = C.S, C.KC, C.HQL, C.KVL, C.NT, C.G, C.WD
    H2 = S // 2
    xT16, wqkv, wv_in, wo_in = io["xT16"], io["wqkv"], io["wv_in"], io["wo_in"]
    cosT_in, sinT_in, masks_in, wnorm_in = io["cosT"], io["sinT"], io["masks"], io["wnorm"]
    out_sh = io["out_sh"]
    chunks = head_chunks(C)

    from contextlib import ExitStack
    ctx = ExitStack()
    with ctx:
        singles = ctx.enter_context(tc.tile_pool(name="singles", bufs=1))
        dramcc = ctx.enter_context(tc.tile_pool(name="dramcc", bufs=1, space="DRAM"))

        ones16 = singles.tile([128, 1], BF16)
        nc.vector.memset(ones16[:], 1.0)
        ones128 = singles.tile([128, 128], BF16)
        nc.vector.memset(ones128[:], 1.0)
        wnorm_sb = singles.tile([128, HQL + KVL], F32)
        nc.sync.dma_start(wnorm_sb[:], wnorm_in[:])
        cq2 = C.CQ * C.CQ
        eps_q = singles.tile([128, 1], F32)
        nc.vector.memset(eps_q[:], C.EPS / cq2)
        eps_k = singles.tile([128, 1], F32)
        nc.vector.memset(eps_k[:], C.EPS)

        cc_nins = [dramcc.tile([1, 1024], F32, name=f"ccni{st}")
                   for st in range(C.NST)]
        cc_nouts = [dramcc.tile([1, 1024], F32, name=f"ccno{st}")
                    for st in range(C.NST)]
        # one gather per 512-wide s-range (phase B runs g-outer), so phase C
        # consumes s-slabs progressively and never waits on the last
        # collective; 4 gathers keeps the ~40us fixed CC cost per collective
        # well under phase B's span
        NGA = G // 2
        cc_ains = [dramcc.tile([HQL, 128, 512], BF16, name=f"ccag{gi}")
                   for gi in range(NGA)]
        cc_aouts = [dramcc.tile([C.TP, HQL, 128, 512], BF16,
                                name=f"ccaog{gi}") for gi in range(NGA)]

        wvp = ctx.enter_context(tc.tile_pool(name="wvp", bufs=1))
        wv_sb = wvp.tile([128, KC, KVL * 128], BF16)
        nc.gpsimd.dma_start(wv_sb[:], wv_in[:])

        with (
            tc.tile_pool(name="xqp", bufs=1) as xq_pool,
            tc.tile_pool(name="xkp", bufs=1) as xk_pool,
            tc.tile_pool(name="vp", bufs=1) as v_pool,
        ):
            xq_sb = xq_pool.tile([128, HQL, S], BF16)
            xk_sb = xk_pool.tile([128, KVL, S], BF16)
            v_sb = v_pool.tile([128, NT, KVL, 128], BF16)

            # ---- phase A: q/k projections (transposed out) + norm
            # partials + V projection, all per 512-wide s-tile. V reuses the
            # same xt tiles as stationary, so x is loaded exactly once.
            # Rope chains are emitted with a 2-tile lag so each slice's
            # AllReduce has completed long before its chain runs; slices 0-1
            # rope on the vector engine during A itself.
            KCH = KC // 2
            with (
                tc.tile_pool(name="xt1", bufs=2) as xt1,
                tc.tile_pool(name="wst", bufs=2) as wst,
                tc.tile_pool(name="sqp", bufs=2) as sqp,
                tc.tile_pool(name="trow", bufs=1) as trow,
                tc.tile_pool(name="tabsQ", bufs=1) as tabs_q,
                tc.tile_pool(name="rowsp", bufs=1) as rowsp,
                tc.tile_pool(name="ropep", bufs=1) as ropep,
                tc.tile_pool(name="psA", bufs=3, space="PSUM") as psA,
                tc.tile_pool(name="psN", bufs=1, space="PSUM") as psN,
                tc.tile_pool(name="psV", bufs=3, space="PSUM") as psV,
            ):

                def emit_rchain(st):
                    sl = slice(st * 512, (st + 1) * 512)
                    # broadcast raw partial sums, then r = exp(-0.5*ln(ax+b))
                    rowraw = rowsp.tile([1, 1024], F32, tag="rowraw")
                    nc.gpsimd.dma_start(rowraw[:], cc_nouts[st][:])
                    rq_b = rowsp.tile([128, 512], F32, tag="rqb")
                    rk_b = rowsp.tile([128, 512], F32, tag="rkb")
                    nc.gpsimd.partition_broadcast(rk_b[:],
                                                  rowraw[0:1, 512:1024])
                    nc.gpsimd.partition_broadcast(rq_b[:],
                                                  rowraw[0:1, 0:512])
                    nc.scalar.activation(rk_b[:], rk_b[:], AF.Ln,
                                         scale=1.0 / (C.HKV * 128),
                                         bias=eps_k[:])
                    nc.scalar.activation(rk_b[:], rk_b[:], AF.Exp, scale=-0.5)
                    nc.scalar.activation(rq_b[:], rq_b[:], AF.Ln,
                                         scale=1.0 / (C.DIM * cq2),
                                         bias=eps_q[:])
                    nc.scalar.activation(rq_b[:], rq_b[:], AF.Exp, scale=-0.5)
                    cosq = tabs_q.tile([128, 512], BF16, tag="cosq")
                    sinq = tabs_q.tile([128, 512], BF16, tag="sinq")
                    cosk = tabs_q.tile([128, 512], BF16, tag="cosk")
                    sink = tabs_q.tile([128, 512], BF16, tag="sink")
                    nc.gpsimd.dma_start(cosk[:], cosT_in[:, sl])
                    nc.gpsimd.dma_start(sink[:], sinT_in[:, sl])
                    nc.gpsimd.dma_start(cosq[:], cosT_in[:, sl])
                    nc.gpsimd.dma_start(sinq[:], sinT_in[:, sl])
                    nc.vector.tensor_mul(cosk[:], cosk[:], rk_b[:])
                    nc.vector.tensor_mul(sink[:], sink[:], rk_b[:])
                    nc.vector.tensor_mul(cosq[:], cosq[:], rq_b[:])
                    nc.vector.tensor_mul(sinq[:], sinq[:], rq_b[:])
                    rotk = ropep.tile([128, KVL, 512], BF16, tag="rotk")
                    rotq = ropep.tile([128, HQL, 512], BF16, tag="rotq")
                    nc.gpsimd.dma_start(rotk[0:64], xk_sb[64:128, :, sl])
                    nc.gpsimd.dma_start(rotk[64:128], xk_sb[0:64, :, sl])
                    nc.gpsimd.dma_start(rotq[0:64], xq_sb[64:128, :, sl])
                    nc.gpsimd.dma_start(rotq[64:128], xq_sb[0:64, :, sl])
                    # fully in-place on vector: rot *= sin; x *= cos; x += rot
                    nc.vector.tensor_mul(
                        rotk[:], rotk[:],
                        sink[:, None, :].to_broadcast((128, KVL, 512)))
                    nc.vector.tensor_mul(
                        xk_sb[:, :, sl], xk_sb[:, :, sl],
                        cosk[:, None, :].to_broadcast((128, KVL, 512)))
                    nc.vector.tensor_add(xk_sb[:, :, sl], xk_sb[:, :, sl],
                                         rotk[:])
                    nc.vector.tensor_mul(
                        rotq[:], rotq[:],
                        sinq[:, None, :].to_broadcast((128, HQL, 512)))
                    nc.vector.tensor_mul(
                        xq_sb[:, :, sl], xq_sb[:, :, sl],
                        cosq[:, None, :].to_broadcast((128, HQL, 512)))
                    nc.vector.tensor_add(xq_sb[:, :, sl], xq_sb[:, :, sl],
                                         rotq[:])

                for st in range(C.NST):
                    if st >= 2:
                        emit_rchain(st - 2)
                    s0 = st * 512
                    xt_a = xt1.tile([128, KCH, 512], BF16, tag="xta")
                    xt_b = xt1.tile([128, KCH, 512], BF16, tag="xtb")
                    nc.sync.dma_start(
                        xt_a[:],
                        xT16[:KCH, :, s0:s0 + 512].rearrange("kc p s -> p kc s"))
                    nc.sync.dma_start(
                        xt_b[:],
                        xT16[KCH:, :, s0:s0 + 512].rearrange("kc p s -> p kc s"))
                    ps_nq = psN.tile([1, 512], F32, tag="nq")
                    ps_nk = psN.tile([1, 512], F32, tag="nk")
                    for hc in range(C.HCL):
                        w_sb = wst.tile([128, KC, 128], BF16, tag="w")
                        nc.sync.dma_start(w_sb[:], wqkv[hc])
                        ps = psA.tile([128, 512], F32, tag="proj")
                        for kc in range(KC):
                            xsrc = xt_a if kc < KCH else xt_b
                            nc.tensor.matmul(ps[:], w_sb[:, kc, :],
                                             xsrc[:, kc % KCH, :],
                                             start=(kc == 0), stop=(kc == KC - 1))
                        if hc < HQL:
                            dest = xq_sb[:, hc, s0:s0 + 512]
                        else:
                            dest = xk_sb[:, hc - HQL, s0:s0 + 512]
                        nc.vector.tensor_scalar_mul(dest, ps[:],
                                                    wnorm_sb[:, hc:hc + 1])
                        sq = sqp.tile([128, 512], BF16, tag="sq")
                        nc.scalar.activation(sq[:], ps[:], AF.Square)
                        tgt = ps_nq if hc < HQL else ps_nk
                        first = (hc == 0) or (hc == HQL)
                        last = (hc == HQL - 1) or (hc == C.HCL - 1)
                        nc.tensor.matmul(tgt[:], ones16[:], sq[:],
                                         start=first, stop=last)
                    rq_t = trow.tile([1, 512], F32, tag="rq")
                    rk_t = trow.tile([1, 512], F32, tag="rk")
                    nc.vector.tensor_copy(rq_t[:], ps_nq[:])
                    nc.vector.tensor_copy(rk_t[:], ps_nk[:])
                    nc.sync.dma_start(cc_nins[st][0:1, 0:512], rq_t[:])
                    nc.sync.dma_start(cc_nins[st][0:1, 512:1024], rk_t[:])
                    # per-s-tile AllReduce of norm partials: overlaps A compute
                    nc.gpsimd.collective_compute(
                        "AllReduce", ALU.add, replica_groups=C.groups,
                        ins=[cc_nins[st].opt()], outs=[cc_nouts[st].opt()])
                    # V projection for this s-tile, x chunks as stationary
                    for tc4 in range(4):
                        tt = st * 4 + tc4
                        psv = psV.tile([128, KVL * 128], F32, tag="v")
                        for kc in range(KC):
                            xsrc = xt_a if kc < KCH else xt_b
                            nc.tensor.matmul(
                                psv[:],
                                xsrc[:, kc % KCH,
                                     tc4 * 128:(tc4 + 1) * 128],
                                wv_sb[:, kc, :],
                                start=(kc == 0), stop=(kc == KC - 1))
                        nc.scalar.copy(v_sb[:, tt, :, :], psv[:])

                emit_rchain(C.NST - 2)
                emit_rchain(C.NST - 1)

            # ---- phase B: banded attention, software-pipelined --------------
            with (
                tc.tile_pool(name="attnp", bufs=1) as attnp,
                tc.tile_pool(name="maskp", bufs=1) as maskp,
                tc.tile_pool(name="expp", bufs=4) as expp,
                tc.tile_pool(name="bmisc", bufs=3) as bmisc,
                tc.tile_pool(name="psSC", bufs=2, space="PSUM") as psSC,
                tc.tile_pool(name="psAT", bufs=2, space="PSUM") as psAT,
                tc.tile_pool(name="psDN", bufs=2, space="PSUM") as psDN,
            ):
                attnT = attnp.tile([128, HQL, S], BF16)
                masks_sb = maskp.tile([128, 4, 256], F32)
                nc.sync.dma_start(masks_sb[:], masks_in[:])
                off2m = {0: 0, 1: 1, WD: 2, WD + 1: 3}

                for g in range(G):
                    jlo, jhi = max(0, 2 * g - WD), 2 * g + 1
                    npairs = (jhi - jlo + 1) // 2
                    for ci, (kv, hs) in enumerate(chunks):
                        w = len(hs)
                        h0 = hs[0]
                        ps_at = psAT.tile([128, 512], F32, tag="at")
                        # ones128 stationary -> denominator lands pre-broadcast
                        # across all 128 partitions (and avoids the 1-wide
                        # stationary pipeline penalty)
                        ps_dn = psDN.tile([128, 512], F32, tag="dn")
                        pend = []  # (ex slice, j) waiting for dn/at emission

                        def drain():
                            for exp_, jp in pend:
                                nc.tensor.matmul(ps_dn[:, :w * 256],
                                                 ones128[:],
                                                 exp_, start=(jp == jlo),
                                                 stop=(jp == jhi))
                                nc.tensor.matmul(ps_at[:, :w * 256],
                                                 v_sb[:, jp, kv, :],
                                                 exp_, start=(jp == jlo),
                                                 stop=(jp == jhi))
                            pend.clear()

                        for p in range(npairs):
                            j0 = jlo + 2 * p
                            ps2 = psSC.tile([128, 1024], F32, tag="sc")
                            for dj in range(2):
                                j = j0 + dj
                                o = dj * 512
                                nc.tensor.matmul(
                                    ps2[:, o:o + w * 256].rearrange(
                                        "p (w s) -> p w s", w=w),
                                    xk_sb[:, kv, j * 128:(j + 1) * 128],
                                    xq_sb[:, h0:h0 + w, g * 256:(g + 1) * 256],
                                    start=True, stop=True)
                            # consume the previous pair while this pair's exp
                            # runs -> the PE never waits on the scalar engine
                            drain()
                            for dj in range(2):
                                j = j0 + dj
                                m = off2m.get(jhi - j)
                                if m is not None:
                                    o = dj * 512
                                    nc.vector.tensor_add(
                                        ps2[:, o:o + w * 256].rearrange(
                                            "p (w s) -> p w s", w=w),
                                        ps2[:, o:o + w * 256].rearrange(
                                            "p (w s) -> p w s", w=w),
                                        masks_sb[:, m, None, :].to_broadcast(
                                            (128, w, 256)))
                            ex2 = expp.tile([128, 1024], BF16, tag="ex")
                            nc.scalar.activation(
                                ex2.rearrange("p (j s) -> p j s",
                                              j=2)[:, :, :w * 256],
                                ps2.rearrange("p (j s) -> p j s",
                                              j=2)[:, :, :w * 256],
                                AF.Exp)
                            pend.append((ex2[:, 0:w * 256], j0))
                            pend.append((ex2[:, 512:512 + w * 256], j0 + 1))
                        drain()
                        den_b = bmisc.tile([128, 512], F32, tag="denb")
                        nc.vector.reciprocal_approx_fast(
                            out=den_b[:, :w * 256], in_=ps_dn[:, :w * 256])
                        nc.vector.tensor_mul(
                            attnT[:, h0:h0 + w, g * 256:(g + 1) * 256],
                            ps_at[:, :w * 256].rearrange(
                                "p (w s) -> p w s", w=w),
                            den_b[:, :w * 256].rearrange(
                                "p (w s) -> p w s", w=w))
                    # all heads for this s-range done: ship + gather the slab
                    # every second g. gpsimd is otherwise idle in B, so the
                    # collective's short input wait can't block anything.
                    if g % 2 == 1:
                        gi = g // 2
                        nc.sync.dma_start(
                            cc_ains[gi].rearrange("h p s -> p h s"),
                            attnT[:, :, (g - 1) * 256:(g + 1) * 256])
                        nc.gpsimd.collective_compute(
                            "AllGather", ALU.bypass,
                            replica_groups=C.groups,
                            ins=[cc_ains[gi].opt()], outs=[cc_aouts[gi].opt()])

        # ---- phase C: output projection ----------------------------------
        # lhs loads pull each gathered 512-wide slab whole (1KB-contiguous
        # per-partition segments -> cheap descriptors); wo is resident in
        # three column panels, smallest loaded first so the PE barely waits.
        with (
            tc.tile_pool(name="wop", bufs=1) as wop,
            tc.tile_pool(name="lhsp", bufs=2) as lhsp,
            tc.tile_pool(name="outp", bufs=3) as outp,
            tc.tile_pool(name="psO", bufs=3, space="PSUM") as psO,
        ):
            col_ts = [(1024, 256), (0, 512), (512, 512)]
            wo_ps = []
            for ci, (c0, wdt) in enumerate(col_ts):
                wo_p = wop.tile([128, C.HQ, wdt], BF16, tag=f"wop{ci}",
                                name=f"wop{ci}")
                nc.sync.dma_start(wo_p[:], wo_in[:, :, c0:c0 + wdt])
                wo_ps.append(wo_p)
            for blk in range(NGA):
                lhs = lhsp.tile([128, C.HQ, 512], BF16, tag="lhs")
                nc.sync.dma_start(
                    lhs[:],
                    cc_aouts[blk].rearrange("r h p s -> p (r h) s"))
                for sbl in range(4):
                    sb = blk * 4 + sbl
                    off = sbl * 128
                    for ci, (c0, wdt) in enumerate(col_ts):
                        ps_o = psO.tile([128, 512], F32, tag="o")
                        for slot in range(C.HQ):
                            nc.tensor.matmul(
                                ps_o[:, :wdt],
                                lhs[:, slot, off:off + 128],
                                wo_ps[ci][:, slot, :],
                                start=(slot == 0),
                                stop=(slot == C.HQ - 1))
                        oro = outp.tile([128, 512], F32, tag="oro")
                        nc.vector.tensor_copy(oro[:, :wdt], ps_o[:, :wdt])
                        nc.sync.dma_start(
                            out_sh[sb * 128:(sb + 1) * 128, c0:c0 + wdt],
                            oro[:, :wdt])


def build_program(C):
    nc = bacc.Bacc("TRN2", target_bir_lowering=False, debug=False,
                   num_devices=C.NC)
    io = {
        "xT16": nc.dram_tensor("xT16", [C.KC, 128, C.S], BF16, kind="ExternalInput").ap(),
        "wqkv": nc.dram_tensor("wqkv", [C.HCL, 128, C.KC, 128], BF16,
                               kind="ExternalInput").ap(),
        "wv_in": nc.dram_tensor("wv_in", [128, C.KC, C.KVL * 128], BF16,
                                kind="ExternalInput").ap(),
        "wo_in": nc.dram_tensor("wo_in", [128, C.HQ, C.COLS], BF16,
                                kind="ExternalInput").ap(),
        "cosT": nc.dram_tensor("cosT", [128, C.S], BF16, kind="ExternalInput").ap(),
        "sinT": nc.dram_tensor("sinT", [128, C.S], BF16, kind="ExternalInput").ap(),
        "masks": nc.dram_tensor("masks", [128, 4, 256], F32, kind="ExternalInput").ap(),
        "wnorm": nc.dram_tensor("wnorm", [128, C.HQL + C.KVL], F32,
                                kind="ExternalInput").ap(),
        "out_sh": nc.dram_tensor("out_sh", [C.S, C.COLS], F32,
                                 kind="ExternalOutput").ap(),
    }
    with tile.TileContext(nc) as tc:
        attention_tile_kernel(tc, C, io)
    nc.compile()
    return nc


def make_masks(mask_np, C):
    """4 mask tiles [t,s-pair] for offsets {0,1,WD,WD+1}; returns [128,4,256] f32."""
    S, WD, SW = C.S, C.WD, C.SW
    I0 = WD + 1

    def tileT(d):
        i, j = I0, I0 - d
        if 0 <= j < C.NT:
            blk = np.array(mask_np[i * 128:(i + 1) * 128, j * 128:(j + 1) * 128],
                           dtype=np.float64)
        else:
            blk = np.full((128, 128), -np.inf)
        s_idx = np.arange(128)[:, None]
        t_idx = np.arange(128)[None, :]
        dist = 128 * d + s_idx - t_idx
        blk = np.where(dist > SW, -np.inf, blk)
        return np.maximum(blk.T, -1e30).astype(np.float32)   # [t, s]

    tiles = []
    for off in (0, 1, WD, WD + 1):
        dl, dr = off - 1, off
        tiles.append(np.concatenate([tileT(dl), tileT(dr)], axis=1))
    return np.ascontiguousarray(np.stack(tiles, axis=1))      # [128, 4, 256]


def make_core_inputs(inputs, C):
    x = np.asarray(inputs["x"], dtype=np.float32)
    wq = np.asarray(inputs["wq"], dtype=np.float32)
    wk = np.asarray(inputs["wk"], dtype=np.float32)
    wv = np.asarray(inputs["wv"], dtype=np.float32)
    wo = np.asarray(inputs["wo"], dtype=np.float32)
    qw = np.asarray(inputs["q_norm_weight"], dtype=np.float32)
    kw = np.asarray(inputs["k_norm_weight"], dtype=np.float32)
    ch = np.asarray(inputs["cos_half"], dtype=np.float32)
    sh = np.asarray(inputs["sin_half"], dtype=np.float32)
    mask = np.asarray(inputs["mask"], dtype=np.float32)
    assert int(inputs.get("start_pos", 0) or 0) == 0

    cosT = np.ascontiguousarray(
        np.concatenate([ch.T, ch.T], axis=0)).astype(BF16_NP)
    sinT = np.ascontiguousarray(
        np.concatenate([-sh.T, sh.T], axis=0)).astype(BF16_NP)
    masks = make_masks(mask, C)
    KC, HQL, KVL = C.KC, C.HQL, C.KVL

    xT_cache = {}
    for b in range(C.DP):
        xT_cache[b] = np.ascontiguousarray(x[b].T).astype(BF16_NP).reshape(
            C.KC, 128, C.S)
    in_maps = []
    for c in range(C.NC):
        b, q4 = c // C.TP, c % C.TP
        x16 = xT_cache[b]
        wq_s = wq[:, 128 * HQL * q4:128 * HQL * (q4 + 1)]
        wk_s = wk[:, 128 * KVL * q4:128 * KVL * (q4 + 1)]
        wv_s = wv[:, 128 * KVL * q4:128 * KVL * (q4 + 1)]
        wqk = np.concatenate([wq_s, wk_s], axis=1).astype(BF16_NP)
        # [HCL, 128, KC, 128]: per chain, contraction-partition-major
        wqkv_pre = np.ascontiguousarray(
            wqk.reshape(KC, 128, C.HCL, 128).transpose(2, 1, 0, 3))
        wv_pre = np.ascontiguousarray(
            wv_s.astype(BF16_NP).reshape(KC, 128, KVL * 128).transpose(1, 0, 2))
        wo_s = wo[:, C.COLS * q4:C.COLS * (q4 + 1)].astype(BF16_NP)
        wo_pre = np.ascontiguousarray(
            wo_s.reshape(C.HQ, 128, C.COLS).transpose(1, 0, 2))
        wn = np.zeros((128, HQL + KVL), dtype=np.float32)
        for hc in range(HQL):
            g = HQL * q4 + hc
            wn[:, hc] = qw[128 * g:128 * (g + 1)]
        for j in range(KVL):
            g = KVL * q4 + j
            wn[:, HQL + j] = kw[128 * g:128 * (g + 1)]
        in_maps.append({"xT16": x16, "wqkv": wqkv_pre, "wv_in": wv_pre,
                        "wo_in": wo_pre, "cosT": cosT, "sinT": sinT,
                        "masks": masks, "wnorm": wn})
    return in_maps


_CACHED = {}


def run(inputs, C=None, trace=False, stitch=None, trace_cores=None):
    C = C or Cfg()
    key = (C.S, C.DIM, C.HQ, C.HKV, C.TP, C.DP, C.SW)
    if key not in _CACHED:
        _CACHED[key] = build_program(C)
    nc = _CACHED[key]
    in_maps = make_core_inputs(inputs, C)
    if stitch is None:
        stitch = trace
    if trace and trace_cores is None:
        trace_cores = list(range(C.NC))
    res = bass_utils.run_bass_kernel_spmd(
        nc, in_maps, core_ids=list(range(C.NC)), trace=trace,
        stitch_traces=stitch, trace_cores=trace_cores if trace else None)
    out = np.empty((C.DP, C.S, C.DIM), dtype=np.float32)
    for c in range(C.NC):
        b, q4 = c // C.TP, c % C.TP
        out[b, :, C.COLS * q4:C.COLS * (q4 + 1)] = res.results[c]["out_sh"]
    return out, res


def kernel(**inputs) -> np.ndarray:
    out, _ = run(inputs)
    return out


# revision 49
# speedup vs baseline: 1.0947x; 1.0055x over previous
"""Sparse (sliding-window) GQA attention prefill kernel for 8 Trainium2 cores.

Sharding: TP=4 over KV heads (2 KV heads + 10 Q heads per core) x DP=2 over
batch. Core c: batch = c // 4, shard q4 = c % 4.

Device program (SPMD, identical on all cores; per-core data via inputs):
  A1: xqT/xkT projections in transposed layout ([head_dim, seq]); sum-of-squares
      for the global RMS norm via Square + ones-matmul; per-s-tile AllReduce of
      the norm partials within each batch group (overlaps A1 compute).
  A2: V projection in natural layout ([seq, head_dim]).
  R:  norm rows -> rope tables (bf16, norm factor folded in); batched rope over
      all local heads per 512-slice (vector for q, gpsimd for k), overlapping A2.
  B:  per (head-pair chunk, 2-query-tile group): scoresT = K^T-chunk.T @ qT in
      the sliding band, mask add, exp, ones-matmul denominator, P^T @ V
      accumulation. Software-pipelined: the denominator/PV matmuls for step j
      are emitted after the scores matmul of step j+1 so the PE never waits on
      the exp. Divide on evacuation via broadcast + reciprocal on [128, .].
      attnT shipped in s-halves: AllGather per (chunk, half), issued late so the
      collective's input wait never blocks the gpsimd queue.
  C:  out = attnT.T @ wo col-shard; first s-half rows start as soon as the
      half-0 gathers land.
"""

import sys
import numpy as np

for _p in ("/opt/trn_rl_repo", "/root/.axon_site/_ro/trn_rl_repo"):
    if _p not in sys.path:
        sys.path.insert(0, _p)

import ml_dtypes

import concourse.bass as bass
import concourse.tile as tile
from concourse import bacc, mybir
from concourse import bass_utils

F32 = mybir.dt.float32
BF16 = mybir.dt.bfloat16
BF16_NP = ml_dtypes.bfloat16
AF = mybir.ActivationFunctionType
ALU = mybir.AluOpType


class Cfg:
    def __init__(self, S=2048, DIM=5120, HQ=40, HKV=8, TP=4, DP=2, SW=1024,
                 MSCALE=1.2079441541679836, EPS=1e-6):
        self.S, self.DIM, self.HQ, self.HKV = S, DIM, HQ, HKV
        self.TP, self.DP, self.SW = TP, DP, SW
        self.MSCALE, self.EPS = MSCALE, EPS
        self.D = 128
        self.NC = TP * DP
        self.HQL = HQ // TP          # local q heads
        self.KVL = HKV // TP         # local kv heads
        self.REP = HQ // HKV
        self.KC = DIM // 128         # contraction chunks
        self.NT = S // 128           # seq tiles
        self.G = self.NT // 2        # 2-query-tile groups
        self.WD = SW // 128          # window in tiles
        self.COLS = DIM // TP        # output column shard
        self.HCL = self.HQL + self.KVL  # projection chains with transposed out
        self.NST = S // 512          # 512-wide s-tiles (phase A1)
        self.NST2 = S // 256         # 256-wide s-tiles (phase A2)
        self.CQ = self.D ** -0.5 * MSCALE
        assert self.WD >= 2 and self.NT > self.WD + 1 and self.NT % 2 == 0
        self.groups = [[b * TP + r for r in range(TP)] for b in range(DP)]


def head_chunks(C):
    """Per-kv head pair chunks: [(kv, [h0,h1]), (kv, [h2,h3]), (kv, [h4])...]"""
    out = []
    per = C.HQL // C.KVL
    for kv in range(C.KVL):
        hs = list(range(kv * per, (kv + 1) * per))
        i = 0
        while i < len(hs):
            out.append((kv, hs[i:i + 2]))
            i += 2
    return out


def attention_tile_kernel(tc, C, io):
    nc = tc.nc
    S, KC, HQL, KVL, NT, G, WD = C.S, C.KC, C.HQL, C.KVL, C.NT, C.G, C.WD
    H2 = S // 2
    xT16, wqkv, wv_in, wo_in = io["xT16"], io["wqkv"], io["wv_in"], io["wo_in"]
    tabqc_in, tabqs_in = io["tabqc"], io["tabqs"]
    tabkc_in, tabks_in = io["tabkc"], io["tabks"]
    masks_in, wnorm_in = io["masks"], io["wnorm"]
    out_sh = io["out_sh"]
    chunks = head_chunks(C)

    from contextlib import ExitStack
    ctx = ExitStack()
    with ctx:
        singles = ctx.enter_context(tc.tile_pool(name="singles", bufs=1))
        dramcc = ctx.enter_context(tc.tile_pool(name="dramcc", bufs=1, space="DRAM"))

        ones16 = singles.tile([128, 1], BF16)
        nc.vector.memset(ones16[:], 1.0)
        ones128 = singles.tile([128, 128], BF16)
        nc.vector.memset(ones128[:], 1.0)
        wnorm_sb = singles.tile([128, HQL + KVL], F32)
        nc.sync.dma_start(wnorm_sb[:], wnorm_in[:])

        cc_nins = [dramcc.tile([1, 1024], F32, name=f"ccni{st}")
                   for st in range(C.NST)]
        cc_nouts = [dramcc.tile([1, 1024], F32, name=f"ccno{st}")
                    for st in range(C.NST)]
        # one gather per 512-wide s-range (phase B runs g-outer), so phase C
        # consumes s-slabs progressively and never waits on the last
        # collective; 4 gathers keeps the ~40us fixed CC cost per collective
        # well under phase B's span
        NGA = G // 2
        cc_ains = [dramcc.tile([HQL, 128, 512], BF16, name=f"ccag{gi}")
                   for gi in range(NGA)]
        cc_aouts = [dramcc.tile([C.TP, HQL, 128, 512], BF16,
                                name=f"ccaog{gi}") for gi in range(NGA)]

        wvp = ctx.enter_context(tc.tile_pool(name="wvp", bufs=1))
        wv_sb = wvp.tile([128, KC, KVL * 128], BF16)
        nc.gpsimd.dma_start(wv_sb[:], wv_in[:])

        with (
            tc.tile_pool(name="xqp", bufs=1) as xq_pool,
            tc.tile_pool(name="xkp", bufs=1) as xk_pool,
            tc.tile_pool(name="vp", bufs=1) as v_pool,
            tc.tile_pool(name="tabsQ", bufs=1) as tabs_q,
            tc.tile_pool(name="rowsp", bufs=1) as rowsp,
            tc.tile_pool(name="ropep", bufs=1) as ropep,
        ):
            xq_sb = xq_pool.tile([128, HQL, S], BF16)
            xk_sb = xk_pool.tile([128, KVL, S], BF16)
            v_sb = v_pool.tile([128, NT, KVL, 128], BF16)

            # ---- phase A: q/k projections (transposed out) + norm
            # partials + V projection, all per 512-wide s-tile. V reuses the
            # same xt tiles as stationary, so x is loaded exactly once.
            # Rope chains are emitted with a 2-tile lag so each slice's
            # AllReduce has completed long before its chain runs; slices 0-1
            # rope on the vector engine during A itself.
            KCH = KC // 2
            with (
                tc.tile_pool(name="xt1", bufs=2) as xt1,
                tc.tile_pool(name="wst", bufs=2) as wst,
                tc.tile_pool(name="sqp", bufs=2) as sqp,
                tc.tile_pool(name="trow", bufs=1) as trow,
                tc.tile_pool(name="psA", bufs=3, space="PSUM") as psA,
                tc.tile_pool(name="psN", bufs=1, space="PSUM") as psN,
                tc.tile_pool(name="psV", bufs=3, space="PSUM") as psV,
            ):

                def emit_rchain(st):
                    sl = slice(st * 512, (st + 1) * 512)
                    # r = raw^-0.5 for both rows in one Ln/Exp pass (norm
                    # constants are folded into the host-prescaled tables)
                    rowraw = rowsp.tile([1, 1024], F32, tag="rowraw")
                    nc.gpsimd.dma_start(rowraw[:], cc_nouts[st][:])
                    rb = rowsp.tile([128, 1024], F32, tag="rb")
                    nc.gpsimd.partition_broadcast(rb[:], rowraw[:])
                    nc.scalar.activation(rb[:], rb[:], AF.Ln)
                    nc.scalar.activation(rb[:], rb[:], AF.Exp, scale=-0.5)
                    cosq = tabs_q.tile([128, 512], BF16, tag="cosq")
                    sinq = tabs_q.tile([128, 512], BF16, tag="sinq")
                    cosk = tabs_q.tile([128, 512], BF16, tag="cosk")
                    sink = tabs_q.tile([128, 512], BF16, tag="sink")
                    nc.gpsimd.dma_start(cosk[:], tabkc_in[:, sl])
                    nc.gpsimd.dma_start(sink[:], tabks_in[:, sl])
                    nc.gpsimd.dma_start(cosq[:], tabqc_in[:, sl])
                    nc.gpsimd.dma_start(sinq[:], tabqs_in[:, sl])
                    nc.vector.tensor_mul(cosk[:], cosk[:], rb[:, 512:1024])
                    nc.vector.tensor_mul(sink[:], sink[:], rb[:, 512:1024])
                    nc.vector.tensor_mul(cosq[:], cosq[:], rb[:, 0:512])
                    nc.vector.tensor_mul(sinq[:], sinq[:], rb[:, 0:512])
                    rotk = ropep.tile([128, KVL, 512], BF16, tag="rotk")
                    rotq = ropep.tile([128, HQL, 512], BF16, tag="rotq")
                    nc.gpsimd.dma_start(rotk[0:64], xk_sb[64:128, :, sl])
                    nc.gpsimd.dma_start(rotk[64:128], xk_sb[0:64, :, sl])
                    nc.gpsimd.dma_start(rotq[0:64], xq_sb[64:128, :, sl])
                    nc.gpsimd.dma_start(rotq[64:128], xq_sb[0:64, :, sl])
                    # fully in-place on vector: rot *= sin; x *= cos; x += rot
                    nc.vector.tensor_mul(
                        rotk[:], rotk[:],
                        sink[:, None, :].to_broadcast((128, KVL, 512)))
                    nc.vector.tensor_mul(
                        xk_sb[:, :, sl], xk_sb[:, :, sl],
                        cosk[:, None, :].to_broadcast((128, KVL, 512)))
                    nc.vector.tensor_add(xk_sb[:, :, sl], xk_sb[:, :, sl],
                                         rotk[:])
                    nc.vector.tensor_mul(
                        rotq[:], rotq[:],
                        sinq[:, None, :].to_broadcast((128, HQL, 512)))
                    nc.vector.tensor_mul(
                        xq_sb[:, :, sl], xq_sb[:, :, sl],
                        cosq[:, None, :].to_broadcast((128, HQL, 512)))
                    nc.vector.tensor_add(xq_sb[:, :, sl], xq_sb[:, :, sl],
                                         rotq[:])

                for st in range(C.NST):
                    if st >= 2:
                        emit_rchain(st - 2)
                    s0 = st * 512
                    xt_a = xt1.tile([128, KCH, 512], BF16, tag="xta")
                    xt_b = xt1.tile([128, KCH, 512], BF16, tag="xtb")
                    nc.sync.dma_start(
                        xt_a[:],
                        xT16[:KCH, :, s0:s0 + 512].rearrange("kc p s -> p kc s"))
                    nc.sync.dma_start(
                        xt_b[:],
                        xT16[KCH:, :, s0:s0 + 512].rearrange("kc p s -> p kc s"))
                    ps_nq = psN.tile([1, 512], F32, tag="nq")
                    ps_nk = psN.tile([1, 512], F32, tag="nk")
                    for hc in range(C.HCL):
                        w_sb = wst.tile([128, KC, 128], BF16, tag="w")
                        nc.sync.dma_start(w_sb[:], wqkv[hc])
                        ps = psA.tile([128, 512], F32, tag="proj")
                        for kc in range(KC):
                            xsrc = xt_a if kc < KCH else xt_b
                            nc.tensor.matmul(ps[:], w_sb[:, kc, :],
                                             xsrc[:, kc % KCH, :],
                                             start=(kc == 0), stop=(kc == KC - 1))
                        if hc < HQL:
                            dest = xq_sb[:, hc, s0:s0 + 512]
                        else:
                            dest = xk_sb[:, hc - HQL, s0:s0 + 512]
                        nc.vector.tensor_scalar_mul(dest, ps[:],
                                                    wnorm_sb[:, hc:hc + 1])
                        sq = sqp.tile([128, 512], BF16, tag="sq")
                        nc.scalar.activation(sq[:], ps[:], AF.Square)
                        tgt = ps_nq if hc < HQL else ps_nk
                        first = (hc == 0) or (hc == HQL)
                        last = (hc == HQL - 1) or (hc == C.HCL - 1)
                        nc.tensor.matmul(tgt[:], ones16[:], sq[:],
                                         start=first, stop=last)
                    rq_t = trow.tile([1, 512], F32, tag="rq")
                    rk_t = trow.tile([1, 512], F32, tag="rk")
                    nc.vector.tensor_copy(rq_t[:], ps_nq[:])
                    nc.vector.tensor_copy(rk_t[:], ps_nk[:])
                    nc.sync.dma_start(cc_nins[st][0:1, 0:512], rq_t[:])
                    nc.sync.dma_start(cc_nins[st][0:1, 512:1024], rk_t[:])
                    # per-s-tile AllReduce of norm partials: overlaps A compute
                    nc.gpsimd.collective_compute(
                        "AllReduce", ALU.add, replica_groups=C.groups,
                        ins=[cc_nins[st].opt()], outs=[cc_nouts[st].opt()])
                    # V projection for this s-tile, x chunks as stationary
                    for tc4 in range(4):
                        tt = st * 4 + tc4
                        psv = psV.tile([128, KVL * 128], F32, tag="v")
                        for kc in range(KC):
                            xsrc = xt_a if kc < KCH else xt_b
                            nc.tensor.matmul(
                                psv[:],
                                xsrc[:, kc % KCH,
                                     tc4 * 128:(tc4 + 1) * 128],
                                wv_sb[:, kc, :],
                                start=(kc == 0), stop=(kc == KC - 1))
                        nc.scalar.copy(v_sb[:, tt, :, :], psv[:])

                emit_rchain(C.NST - 2)
                last_rchain = C.NST - 1

            # ---- phase B: banded attention, software-pipelined --------------
            with (
                tc.tile_pool(name="attnp", bufs=1) as attnp,
                tc.tile_pool(name="maskp", bufs=1) as maskp,
                tc.tile_pool(name="expp", bufs=4) as expp,
                tc.tile_pool(name="bmisc", bufs=3) as bmisc,
                tc.tile_pool(name="psSC", bufs=2, space="PSUM") as psSC,
                tc.tile_pool(name="psAT", bufs=2, space="PSUM") as psAT,
                tc.tile_pool(name="psDN", bufs=2, space="PSUM") as psDN,
            ):
                attnT = attnp.tile([128, HQL, S], BF16)
                masks_sb = maskp.tile([128, 4, 256], F32)
                nc.sync.dma_start(masks_sb[:], masks_in[:])
                off2m = {0: 0, 1: 1, WD: 2, WD + 1: 3}

                for g in range(G):
                    jlo, jhi = max(0, 2 * g - WD), 2 * g + 1
                    npairs = (jhi - jlo + 1) // 2
                    for ci, (kv, hs) in enumerate(chunks):
                        w = len(hs)
                        h0 = hs[0]
                        ps_at = psAT.tile([128, 512], F32, tag="at")
                        # ones128 stationary -> denominator lands pre-broadcast
                        # across all 128 partitions (and avoids the 1-wide
                        # stationary pipeline penalty)
                        ps_dn = psDN.tile([128, 512], F32, tag="dn")
                        pend = []  # (ex slice, j) waiting for dn/at emission

                        def drain():
                            for exp_, jp in pend:
                                nc.tensor.matmul(ps_dn[:, :w * 256],
                                                 ones128[:],
                                                 exp_, start=(jp == jlo),
                                                 stop=(jp == jhi))
                                nc.tensor.matmul(ps_at[:, :w * 256],
                                                 v_sb[:, jp, kv, :],
                                                 exp_, start=(jp == jlo),
                                                 stop=(jp == jhi))
                            pend.clear()

                        for p in range(npairs):
                            j0 = jlo + 2 * p
                            ps2 = psSC.tile([128, 1024], F32, tag="sc")
                            for dj in range(2):
                                j = j0 + dj
                                o = dj * 512
                                nc.tensor.matmul(
                                    ps2[:, o:o + w * 256].rearrange(
                                        "p (w s) -> p w s", w=w),
                                    xk_sb[:, kv, j * 128:(j + 1) * 128],
                                    xq_sb[:, h0:h0 + w, g * 256:(g + 1) * 256],
                                    start=True, stop=True)
                            # consume the previous pair while this pair's exp
                            # runs -> the PE never waits on the scalar engine
                            drain()
                            for dj in range(2):
                                j = j0 + dj
                                m = off2m.get(jhi - j)
                                if m is not None:
                                    o = dj * 512
                                    nc.vector.tensor_add(
                                        ps2[:, o:o + w * 256].rearrange(
                                            "p (w s) -> p w s", w=w),
                                        ps2[:, o:o + w * 256].rearrange(
                                            "p (w s) -> p w s", w=w),
                                        masks_sb[:, m, None, :].to_broadcast(
                                            (128, w, 256)))
                            ex2 = expp.tile([128, 1024], BF16, tag="ex")
                            nc.scalar.activation(
                                ex2.rearrange("p (j s) -> p j s",
                                              j=2)[:, :, :w * 256],
                                ps2.rearrange("p (j s) -> p j s",
                                              j=2)[:, :, :w * 256],
                                AF.Exp)
                            pend.append((ex2[:, 0:w * 256], j0))
                            pend.append((ex2[:, 512:512 + w * 256], j0 + 1))
                        drain()
                        den_b = bmisc.tile([128, 512], F32, tag="denb")
                        nc.vector.reciprocal_approx_fast(
                            out=den_b[:, :w * 256], in_=ps_dn[:, :w * 256])
                        nc.vector.tensor_mul(
                            attnT[:, h0:h0 + w, g * 256:(g + 1) * 256],
                            ps_at[:, :w * 256].rearrange(
                                "p (w s) -> p w s", w=w),
                            den_b[:, :w * 256].rearrange(
                                "p (w s) -> p w s", w=w))
                    # all heads for this s-range done: ship + gather the slab
                    # every second g. gpsimd is otherwise idle in B, so the
                    # collective's short input wait can't block anything.
                    if g % 2 == 1:
                        gi = g // 2
                        nc.sync.dma_start(
                            cc_ains[gi].rearrange("h p s -> p h s"),
                            attnT[:, :, (g - 1) * 256:(g + 1) * 256])
                        nc.gpsimd.collective_compute(
                            "AllGather", ALU.bypass,
                            replica_groups=C.groups,
                            ins=[cc_ains[gi].opt()], outs=[cc_aouts[gi].opt()])
                    if g == 1:
                        # last slice's rope: its AllReduce lands well after A
                        # ends; B only touches slice 3 from g=6 onward.
                        emit_rchain(last_rchain)

        # ---- phase C: output projection ----------------------------------
        # wo resident in three column panels; the small panel and the first
        # lhs sub-slice load first so the PE starts ~12us after B. lhs slabs
        # load whole (1KB-contiguous segments -> cheap descriptors).
        with (
            tc.tile_pool(name="wop", bufs=1) as wop,
            tc.tile_pool(name="lhsp", bufs=2) as lhsp,
            tc.tile_pool(name="outp", bufs=3) as outp,
            tc.tile_pool(name="psO", bufs=3, space="PSUM") as psO,
        ):
            col_ts = [(1024, 256), (0, 512), (512, 512)]
            wo_ps = [wop.tile([128, C.HQ, wdt], BF16, tag=f"wop{ci}",
                              name=f"wop{ci}")
                     for ci, (c0, wdt) in enumerate(col_ts)]
            nc.sync.dma_start(wo_ps[0][:], wo_in[:, :, 1024:1280])

            def c_block(blk, lhs, cis):
                for ci in cis:
                    c0, wdt = col_ts[ci]
                    for sbl in range(4):
                        sb = blk * 4 + sbl
                        off = sbl * 128
                        ps_o = psO.tile([128, 512], F32, tag="o")
                        for slot in range(C.HQ):
                            nc.tensor.matmul(
                                ps_o[:, :wdt],
                                lhs[:, slot, off:off + 128],
                                wo_ps[ci][:, slot, :],
                                start=(slot == 0),
                                stop=(slot == C.HQ - 1))
                        oro = outp.tile([128, 512], F32, tag="oro")
                        nc.vector.tensor_copy(oro[:, :wdt], ps_o[:, :wdt])
                        nc.sync.dma_start(
                            out_sh[sb * 128:(sb + 1) * 128, c0:c0 + wdt],
                            oro[:, :wdt])

            # block 0: lhs in four per-sb pieces so the first chain starts
            # as soon as piece 0 and the small wo panel land
            lhs0 = lhsp.tile([128, C.HQ, 512], BF16, tag="lhs")
            for sbl in range(4):
                nc.sync.dma_start(
                    lhs0[:, :, sbl * 128:(sbl + 1) * 128],
                    cc_aouts[0][:, :, :,
                                sbl * 128:(sbl + 1) * 128].rearrange(
                        "r h p s -> p (r h) s"))
            c_block(0, lhs0, [0])
            nc.sync.dma_start(wo_ps[1][:], wo_in[:, :, 0:512])
            nc.sync.dma_start(wo_ps[2][:], wo_in[:, :, 512:1024])
            c_block(0, lhs0, [1, 2])
            for blk in range(1, NGA):
                lhs = lhsp.tile([128, C.HQ, 512], BF16, tag="lhs")
                nc.sync.dma_start(
                    lhs[:],
                    cc_aouts[blk].rearrange("r h p s -> p (r h) s"))
                c_block(blk, lhs, [0, 1, 2])


def build_program(C):
    nc = bacc.Bacc("TRN2", target_bir_lowering=False, debug=False,
                   num_devices=C.NC)
    io = {
        "xT16": nc.dram_tensor("xT16", [C.KC, 128, C.S], BF16, kind="ExternalInput").ap(),
        "wqkv": nc.dram_tensor("wqkv", [C.HCL, 128, C.KC, 128], BF16,
                               kind="ExternalInput").ap(),
        "wv_in": nc.dram_tensor("wv_in", [128, C.KC, C.KVL * 128], BF16,
                                kind="ExternalInput").ap(),
        "wo_in": nc.dram_tensor("wo_in", [128, C.HQ, C.COLS], BF16,
                                kind="ExternalInput").ap(),
        "tabqc": nc.dram_tensor("tabqc", [128, C.S], BF16, kind="ExternalInput").ap(),
        "tabqs": nc.dram_tensor("tabqs", [128, C.S], BF16, kind="ExternalInput").ap(),
        "tabkc": nc.dram_tensor("tabkc", [128, C.S], BF16, kind="ExternalInput").ap(),
        "tabks": nc.dram_tensor("tabks", [128, C.S], BF16, kind="ExternalInput").ap(),
        "masks": nc.dram_tensor("masks", [128, 4, 256], F32, kind="ExternalInput").ap(),
        "wnorm": nc.dram_tensor("wnorm", [128, C.HQL + C.KVL], F32,
                                kind="ExternalInput").ap(),
        "out_sh": nc.dram_tensor("out_sh", [C.S, C.COLS], F32,
                                 kind="ExternalOutput").ap(),
    }
    with tile.TileContext(nc) as tc:
        attention_tile_kernel(tc, C, io)
    nc.compile()
    return nc


def make_masks(mask_np, C):
    """4 mask tiles [t,s-pair] for offsets {0,1,WD,WD+1}; returns [128,4,256] f32."""
    S, WD, SW = C.S, C.WD, C.SW
    I0 = WD + 1

    def tileT(d):
        i, j = I0, I0 - d
        if 0 <= j < C.NT:
            blk = np.array(mask_np[i * 128:(i + 1) * 128, j * 128:(j + 1) * 128],
                           dtype=np.float64)
        else:
            blk = np.full((128, 128), -np.inf)
        s_idx = np.arange(128)[:, None]
        t_idx = np.arange(128)[None, :]
        dist = 128 * d + s_idx - t_idx
        blk = np.where(dist > SW, -np.inf, blk)
        return np.maximum(blk.T, -1e30).astype(np.float32)   # [t, s]

    tiles = []
    for off in (0, 1, WD, WD + 1):
        dl, dr = off - 1, off
        tiles.append(np.concatenate([tileT(dl), tileT(dr)], axis=1))
    return np.ascontiguousarray(np.stack(tiles, axis=1))      # [128, 4, 256]


def make_core_inputs(inputs, C):
    x = np.asarray(inputs["x"], dtype=np.float32)
    wq = np.asarray(inputs["wq"], dtype=np.float32)
    wk = np.asarray(inputs["wk"], dtype=np.float32)
    wv = np.asarray(inputs["wv"], dtype=np.float32)
    wo = np.asarray(inputs["wo"], dtype=np.float32)
    qw = np.asarray(inputs["q_norm_weight"], dtype=np.float32)
    kw = np.asarray(inputs["k_norm_weight"], dtype=np.float32)
    ch = np.asarray(inputs["cos_half"], dtype=np.float32)
    sh = np.asarray(inputs["sin_half"], dtype=np.float32)
    mask = np.asarray(inputs["mask"], dtype=np.float32)
    assert int(inputs.get("start_pos", 0) or 0) == 0

    cosT = np.ascontiguousarray(np.concatenate([ch.T, ch.T], axis=0))
    sinT = np.ascontiguousarray(np.concatenate([-sh.T, sh.T], axis=0))
    # norm constants folded into the tables: r = raw^-0.5 on device
    cstq = C.CQ * np.sqrt(C.DIM)
    cstk = np.sqrt(C.HKV * 128.0)
    tabqc = (cosT * cstq).astype(BF16_NP)
    tabqs = (sinT * cstq).astype(BF16_NP)
    tabkc = (cosT * cstk).astype(BF16_NP)
    tabks = (sinT * cstk).astype(BF16_NP)
    masks = make_masks(mask, C)
    KC, HQL, KVL = C.KC, C.HQL, C.KVL

    xT_cache = {}
    for b in range(C.DP):
        xT_cache[b] = np.ascontiguousarray(x[b].T).astype(BF16_NP).reshape(
            C.KC, 128, C.S)
    in_maps = []
    for c in range(C.NC):
        b, q4 = c // C.TP, c % C.TP
        x16 = xT_cache[b]
        wq_s = wq[:, 128 * HQL * q4:128 * HQL * (q4 + 1)]
        wk_s = wk[:, 128 * KVL * q4:128 * KVL * (q4 + 1)]
        wv_s = wv[:, 128 * KVL * q4:128 * KVL * (q4 + 1)]
        wqk = np.concatenate([wq_s, wk_s], axis=1).astype(BF16_NP)
        # [HCL, 128, KC, 128]: per chain, contraction-partition-major
        wqkv_pre = np.ascontiguousarray(
            wqk.reshape(KC, 128, C.HCL, 128).transpose(2, 1, 0, 3))
        wv_pre = np.ascontiguousarray(
            wv_s.astype(BF16_NP).reshape(KC, 128, KVL * 128).transpose(1, 0, 2))
        wo_s = wo[:, C.COLS * q4:C.COLS * (q4 + 1)].astype(BF16_NP)
        wo_pre = np.ascontiguousarray(
            wo_s.reshape(C.HQ, 128, C.COLS).transpose(1, 0, 2))
        wn = np.zeros((128, HQL + KVL), dtype=np.float32)
        for hc in range(HQL):
            g = HQL * q4 + hc
            wn[:, hc] = qw[128 * g:128 * (g + 1)]
        for j in range(KVL):
            g = KVL * q4 + j
            wn[:, HQL + j] = kw[128 * g:128 * (g + 1)]
        in_maps.append({"xT16": x16, "wqkv": wqkv_pre, "wv_in": wv_pre,
                        "wo_in": wo_pre, "tabqc": tabqc, "tabqs": tabqs,
                        "tabkc": tabkc, "tabks": tabks,
                        "masks": masks, "wnorm": wn})
    return in_maps


_CACHED = {}


def run(inputs, C=None, trace=False, stitch=None, trace_cores=None):
    C = C or Cfg()
    key = (C.S, C.DIM, C.HQ, C.HKV, C.TP, C.DP, C.SW)
    if key not in _CACHED:
        _CACHED[key] = build_program(C)
    nc = _CACHED[key]
    in_maps = make_core_inputs(inputs, C)
    if stitch is None:
        stitch = trace
    if trace and trace_cores is None:
        trace_cores = list(range(C.NC))
    res = bass_utils.run_bass_kernel_spmd(
        nc, in_maps, core_ids=list(range(C.NC)), trace=trace,
        stitch_traces=stitch, trace_cores=trace_cores if trace else None)
    out = np.empty((C.DP, C.S, C.DIM), dtype=np.float32)
    for c in range(C.NC):
        b, q4 = c // C.TP, c % C.TP
        out[b, :, C.COLS * q4:C.COLS * (q4 + 1)] = res.results[c]["out_sh"]
    return out, res


def kernel(**inputs) -> np.ndarray:
    out, _ = run(inputs)
    return out


# revision 50
# speedup vs baseline: 1.0975x; 1.0026x over previous
"""Sparse (sliding-window) GQA attention prefill kernel for 8 Trainium2 cores.

Sharding: TP=4 over KV heads (2 KV heads + 10 Q heads per core) x DP=2 over
batch. Core c: batch = c // 4, shard q4 = c % 4.

Device program (SPMD, identical on all cores; per-core data via inputs):
  A1: xqT/xkT projections in transposed layout ([head_dim, seq]); sum-of-squares
      for the global RMS norm via Square + ones-matmul; per-s-tile AllReduce of
      the norm partials within each batch group (overlaps A1 compute).
  A2: V projection in natural layout ([seq, head_dim]).
  R:  norm rows -> rope tables (bf16, norm factor folded in); batched rope over
      all local heads per 512-slice (vector for q, gpsimd for k), overlapping A2.
  B:  per (head-pair chunk, 2-query-tile group): scoresT = K^T-chunk.T @ qT in
      the sliding band, mask add, exp, ones-matmul denominator, P^T @ V
      accumulation. Software-pipelined: the denominator/PV matmuls for step j
      are emitted after the scores matmul of step j+1 so the PE never waits on
      the exp. Divide on evacuation via broadcast + reciprocal on [128, .].
      attnT shipped in s-halves: AllGather per (chunk, half), issued late so the
      collective's input wait never blocks the gpsimd queue.
  C:  out = attnT.T @ wo col-shard; first s-half rows start as soon as the
      half-0 gathers land.
"""

import sys
import numpy as np

for _p in ("/opt/trn_rl_repo", "/root/.axon_site/_ro/trn_rl_repo"):
    if _p not in sys.path:
        sys.path.insert(0, _p)

import ml_dtypes

import concourse.bass as bass
import concourse.tile as tile
from concourse import bacc, mybir
from concourse import bass_utils

F32 = mybir.dt.float32
BF16 = mybir.dt.bfloat16
BF16_NP = ml_dtypes.bfloat16
AF = mybir.ActivationFunctionType
ALU = mybir.AluOpType


class Cfg:
    def __init__(self, S=2048, DIM=5120, HQ=40, HKV=8, TP=4, DP=2, SW=1024,
                 MSCALE=1.2079441541679836, EPS=1e-6):
        self.S, self.DIM, self.HQ, self.HKV = S, DIM, HQ, HKV
        self.TP, self.DP, self.SW = TP, DP, SW
        self.MSCALE, self.EPS = MSCALE, EPS
        self.D = 128
        self.NC = TP * DP
        self.HQL = HQ // TP          # local q heads
        self.KVL = HKV // TP         # local kv heads
        self.REP = HQ // HKV
        self.KC = DIM // 128         # contraction chunks
        self.NT = S // 128           # seq tiles
        self.G = self.NT // 2        # 2-query-tile groups
        self.WD = SW // 128          # window in tiles
        self.COLS = DIM // TP        # output column shard
        self.HCL = self.HQL + self.KVL  # projection chains with transposed out
        self.NST = S // 512          # 512-wide s-tiles (phase A1)
        self.NST2 = S // 256         # 256-wide s-tiles (phase A2)
        self.CQ = self.D ** -0.5 * MSCALE
        assert self.WD >= 2 and self.NT > self.WD + 1 and self.NT % 2 == 0
        self.groups = [[b * TP + r for r in range(TP)] for b in range(DP)]


def head_chunks(C):
    """Per-kv head pair chunks: [(kv, [h0,h1]), (kv, [h2,h3]), (kv, [h4])...]"""
    out = []
    per = C.HQL // C.KVL
    for kv in range(C.KVL):
        hs = list(range(kv * per, (kv + 1) * per))
        i = 0
        while i < len(hs):
            out.append((kv, hs[i:i + 2]))
            i += 2
    return out


def attention_tile_kernel(tc, C, io):
    nc = tc.nc
    S, KC, HQL, KVL, NT, G, WD = C.S, C.KC, C.HQL, C.KVL, C.NT, C.G, C.WD
    H2 = S // 2
    xT16, wqkv, wv_in, wo_in = io["xT16"], io["wqkv"], io["wv_in"], io["wo_in"]
    tabqc_in, tabqs_in = io["tabqc"], io["tabqs"]
    tabkc_in, tabks_in = io["tabkc"], io["tabks"]
    masks_in, wnorm_in = io["masks"], io["wnorm"]
    out_sh = io["out_sh"]
    chunks = head_chunks(C)

    from contextlib import ExitStack
    ctx = ExitStack()
    with ctx:
        singles = ctx.enter_context(tc.tile_pool(name="singles", bufs=1))
        dramcc = ctx.enter_context(tc.tile_pool(name="dramcc", bufs=1, space="DRAM"))

        ones16 = singles.tile([128, 1], BF16)
        nc.vector.memset(ones16[:], 1.0)
        ones128 = singles.tile([128, 128], BF16)
        nc.vector.memset(ones128[:], 1.0)
        wnorm_sb = singles.tile([128, HQL + KVL], F32)
        nc.sync.dma_start(wnorm_sb[:], wnorm_in[:])

        cc_nins = [dramcc.tile([1, 1024], F32, name=f"ccni{st}")
                   for st in range(C.NST)]
        cc_nouts = [dramcc.tile([1, 1024], F32, name=f"ccno{st}")
                    for st in range(C.NST)]
        # one gather per 512-wide s-range (phase B runs g-outer), so phase C
        # consumes s-slabs progressively and never waits on the last
        # collective; 4 gathers keeps the ~40us fixed CC cost per collective
        # well under phase B's span
        NGA = G // 2
        cc_ains = [dramcc.tile([HQL, 128, 512], BF16, name=f"ccag{gi}")
                   for gi in range(NGA)]
        cc_aouts = [dramcc.tile([C.TP, HQL, 128, 512], BF16,
                                name=f"ccaog{gi}") for gi in range(NGA)]

        wvp = ctx.enter_context(tc.tile_pool(name="wvp", bufs=1))
        wv_sb = wvp.tile([128, KC, KVL * 128], BF16)
        nc.gpsimd.dma_start(wv_sb[:], wv_in[:])

        with (
            tc.tile_pool(name="xqp", bufs=1) as xq_pool,
            tc.tile_pool(name="xkp", bufs=1) as xk_pool,
            tc.tile_pool(name="vp", bufs=1) as v_pool,
            tc.tile_pool(name="tabsQ", bufs=1) as tabs_q,
            tc.tile_pool(name="rowsp", bufs=1) as rowsp,
            tc.tile_pool(name="ropep", bufs=1) as ropep,
        ):
            xq_sb = xq_pool.tile([128, HQL, S], BF16)
            xk_sb = xk_pool.tile([128, KVL, S], BF16)
            v_sb = v_pool.tile([128, NT, KVL, 128], BF16)

            # ---- phase A: q/k projections (transposed out) + norm
            # partials + V projection, all per 512-wide s-tile. V reuses the
            # same xt tiles as stationary, so x is loaded exactly once.
            # Rope chains are emitted with a 2-tile lag so each slice's
            # AllReduce has completed long before its chain runs; slices 0-1
            # rope on the vector engine during A itself.
            KCH = KC // 2
            with (
                tc.tile_pool(name="xt1", bufs=2) as xt1,
                tc.tile_pool(name="wst", bufs=2) as wst,
                tc.tile_pool(name="sqp", bufs=2) as sqp,
                tc.tile_pool(name="trow", bufs=1) as trow,
                tc.tile_pool(name="psA", bufs=3, space="PSUM") as psA,
                tc.tile_pool(name="psN", bufs=1, space="PSUM") as psN,
                tc.tile_pool(name="psV", bufs=3, space="PSUM") as psV,
            ):

                def emit_rchain(st):
                    sl = slice(st * 512, (st + 1) * 512)
                    # r = raw^-0.5 for both rows in one Ln/Exp pass (norm
                    # constants are folded into the host-prescaled tables)
                    rowraw = rowsp.tile([1, 1024], F32, tag="rowraw")
                    nc.gpsimd.dma_start(rowraw[:], cc_nouts[st][:])
                    rb = rowsp.tile([128, 1024], F32, tag="rb")
                    nc.gpsimd.partition_broadcast(rb[:], rowraw[:])
                    nc.scalar.activation(rb[:], rb[:], AF.Ln)
                    nc.scalar.activation(rb[:], rb[:], AF.Exp, scale=-0.5)
                    cosq = tabs_q.tile([128, 512], BF16, tag="cosq")
                    sinq = tabs_q.tile([128, 512], BF16, tag="sinq")
                    cosk = tabs_q.tile([128, 512], BF16, tag="cosk")
                    sink = tabs_q.tile([128, 512], BF16, tag="sink")
                    nc.gpsimd.dma_start(cosk[:], tabkc_in[:, sl])
                    nc.gpsimd.dma_start(sink[:], tabks_in[:, sl])
                    nc.gpsimd.dma_start(cosq[:], tabqc_in[:, sl])
                    nc.gpsimd.dma_start(sinq[:], tabqs_in[:, sl])
                    nc.vector.tensor_mul(cosk[:], cosk[:], rb[:, 512:1024])
                    nc.vector.tensor_mul(sink[:], sink[:], rb[:, 512:1024])
                    nc.vector.tensor_mul(cosq[:], cosq[:], rb[:, 0:512])
                    nc.vector.tensor_mul(sinq[:], sinq[:], rb[:, 0:512])
                    rotk = ropep.tile([128, KVL, 512], BF16, tag="rotk")
                    rotq = ropep.tile([128, HQL, 512], BF16, tag="rotq")
                    nc.gpsimd.dma_start(rotk[0:64], xk_sb[64:128, :, sl])
                    nc.gpsimd.dma_start(rotk[64:128], xk_sb[0:64, :, sl])
                    nc.gpsimd.dma_start(rotq[0:64], xq_sb[64:128, :, sl])
                    nc.gpsimd.dma_start(rotq[64:128], xq_sb[0:64, :, sl])
                    # fully in-place on vector: rot *= sin; x *= cos; x += rot
                    nc.vector.tensor_mul(
                        rotk[:], rotk[:],
                        sink[:, None, :].to_broadcast((128, KVL, 512)))
                    nc.vector.tensor_mul(
                        xk_sb[:, :, sl], xk_sb[:, :, sl],
                        cosk[:, None, :].to_broadcast((128, KVL, 512)))
                    nc.vector.tensor_add(xk_sb[:, :, sl], xk_sb[:, :, sl],
                                         rotk[:])
                    nc.vector.tensor_mul(
                        rotq[:], rotq[:],
                        sinq[:, None, :].to_broadcast((128, HQL, 512)))
                    nc.vector.tensor_mul(
                        xq_sb[:, :, sl], xq_sb[:, :, sl],
                        cosq[:, None, :].to_broadcast((128, HQL, 512)))
                    nc.vector.tensor_add(xq_sb[:, :, sl], xq_sb[:, :, sl],
                                         rotq[:])

                for st in range(C.NST):
                    if st >= 2:
                        emit_rchain(st - 2)
                    s0 = st * 512
                    xt_a = xt1.tile([128, KCH, 512], BF16, tag="xta")
                    xt_b = xt1.tile([128, KCH, 512], BF16, tag="xtb")
                    nc.sync.dma_start(
                        xt_a[:],
                        xT16[:KCH, :, s0:s0 + 512].rearrange("kc p s -> p kc s"))
                    nc.sync.dma_start(
                        xt_b[:],
                        xT16[KCH:, :, s0:s0 + 512].rearrange("kc p s -> p kc s"))
                    ps_nq = psN.tile([1, 512], F32, tag="nq")
                    ps_nk = psN.tile([1, 512], F32, tag="nk")
                    for hc in range(C.HCL):
                        w_sb = wst.tile([128, KC, 128], BF16, tag="w")
                        nc.sync.dma_start(w_sb[:], wqkv[hc])
                        ps = psA.tile([128, 512], F32, tag="proj")
                        for kc in range(KC):
                            xsrc = xt_a if kc < KCH else xt_b
                            nc.tensor.matmul(ps[:], w_sb[:, kc, :],
                                             xsrc[:, kc % KCH, :],
                                             start=(kc == 0), stop=(kc == KC - 1))
                        if hc < HQL:
                            dest = xq_sb[:, hc, s0:s0 + 512]
                        else:
                            dest = xk_sb[:, hc - HQL, s0:s0 + 512]
                        nc.vector.tensor_scalar_mul(dest, ps[:],
                                                    wnorm_sb[:, hc:hc + 1])
                        sq = sqp.tile([128, 512], BF16, tag="sq")
                        nc.scalar.activation(sq[:], ps[:], AF.Square)
                        tgt = ps_nq if hc < HQL else ps_nk
                        first = (hc == 0) or (hc == HQL)
                        last = (hc == HQL - 1) or (hc == C.HCL - 1)
                        nc.tensor.matmul(tgt[:], ones16[:], sq[:],
                                         start=first, stop=last)
                    rq_t = trow.tile([1, 512], F32, tag="rq")
                    rk_t = trow.tile([1, 512], F32, tag="rk")
                    nc.vector.tensor_copy(rq_t[:], ps_nq[:])
                    nc.vector.tensor_copy(rk_t[:], ps_nk[:])
                    nc.sync.dma_start(cc_nins[st][0:1, 0:512], rq_t[:])
                    nc.sync.dma_start(cc_nins[st][0:1, 512:1024], rk_t[:])
                    # per-s-tile AllReduce of norm partials: overlaps A compute
                    nc.gpsimd.collective_compute(
                        "AllReduce", ALU.add, replica_groups=C.groups,
                        ins=[cc_nins[st].opt()], outs=[cc_nouts[st].opt()])
                    # V projection for this s-tile, x chunks as stationary
                    for tc4 in range(4):
                        tt = st * 4 + tc4
                        psv = psV.tile([128, KVL * 128], F32, tag="v")
                        for kc in range(KC):
                            xsrc = xt_a if kc < KCH else xt_b
                            nc.tensor.matmul(
                                psv[:],
                                xsrc[:, kc % KCH,
                                     tc4 * 128:(tc4 + 1) * 128],
                                wv_sb[:, kc, :],
                                start=(kc == 0), stop=(kc == KC - 1))
                        nc.scalar.copy(v_sb[:, tt, :, :], psv[:])


            # ---- phase B: banded attention, software-pipelined --------------
            with (
                tc.tile_pool(name="attnp", bufs=1) as attnp,
                tc.tile_pool(name="maskp", bufs=1) as maskp,
                tc.tile_pool(name="expp", bufs=4) as expp,
                tc.tile_pool(name="bmisc", bufs=3) as bmisc,
                tc.tile_pool(name="psSC", bufs=2, space="PSUM") as psSC,
                tc.tile_pool(name="psAT", bufs=2, space="PSUM") as psAT,
                tc.tile_pool(name="psDN", bufs=2, space="PSUM") as psDN,
            ):
                attnT = attnp.tile([128, HQL, S], BF16)
                masks_sb = maskp.tile([128, 4, 256], F32)
                nc.sync.dma_start(masks_sb[:], masks_in[:])
                off2m = {0: 0, 1: 1, WD: 2, WD + 1: 3}

                for g in range(G):
                    jlo, jhi = max(0, 2 * g - WD), 2 * g + 1
                    npairs = (jhi - jlo + 1) // 2
                    for ci, (kv, hs) in enumerate(chunks):
                        w = len(hs)
                        h0 = hs[0]
                        ps_at = psAT.tile([128, 512], F32, tag="at")
                        # ones128 stationary -> denominator lands pre-broadcast
                        # across all 128 partitions (and avoids the 1-wide
                        # stationary pipeline penalty)
                        ps_dn = psDN.tile([128, 512], F32, tag="dn")
                        pend = []  # (ex slice, j) waiting for dn/at emission

                        def drain():
                            for exp_, jp in pend:
                                nc.tensor.matmul(ps_dn[:, :w * 256],
                                                 ones128[:],
                                                 exp_, start=(jp == jlo),
                                                 stop=(jp == jhi))
                                nc.tensor.matmul(ps_at[:, :w * 256],
                                                 v_sb[:, jp, kv, :],
                                                 exp_, start=(jp == jlo),
                                                 stop=(jp == jhi))
                            pend.clear()

                        for p in range(npairs):
                            j0 = jlo + 2 * p
                            ps2 = psSC.tile([128, 1024], F32, tag="sc")
                            for dj in range(2):
                                j = j0 + dj
                                o = dj * 512
                                nc.tensor.matmul(
                                    ps2[:, o:o + w * 256].rearrange(
                                        "p (w s) -> p w s", w=w),
                                    xk_sb[:, kv, j * 128:(j + 1) * 128],
                                    xq_sb[:, h0:h0 + w, g * 256:(g + 1) * 256],
                                    start=True, stop=True)
                            # consume the previous pair while this pair's exp
                            # runs -> the PE never waits on the scalar engine
                            drain()
                            for dj in range(2):
                                j = j0 + dj
                                m = off2m.get(jhi - j)
                                if m is not None:
                                    o = dj * 512
                                    nc.vector.tensor_add(
                                        ps2[:, o:o + w * 256].rearrange(
                                            "p (w s) -> p w s", w=w),
                                        ps2[:, o:o + w * 256].rearrange(
                                            "p (w s) -> p w s", w=w),
                                        masks_sb[:, m, None, :].to_broadcast(
                                            (128, w, 256)))
                            ex2 = expp.tile([128, 1024], BF16, tag="ex")
                            nc.scalar.activation(
                                ex2.rearrange("p (j s) -> p j s",
                                              j=2)[:, :, :w * 256],
                                ps2.rearrange("p (j s) -> p j s",
                                              j=2)[:, :, :w * 256],
                                AF.Exp)
                            pend.append((ex2[:, 0:w * 256], j0))
                            pend.append((ex2[:, 512:512 + w * 256], j0 + 1))
                        drain()
                        den_b = bmisc.tile([128, 512], F32, tag="denb")
                        nc.vector.reciprocal_approx_fast(
                            out=den_b[:, :w * 256], in_=ps_dn[:, :w * 256])
                        nc.vector.tensor_mul(
                            attnT[:, h0:h0 + w, g * 256:(g + 1) * 256],
                            ps_at[:, :w * 256].rearrange(
                                "p (w s) -> p w s", w=w),
                            den_b[:, :w * 256].rearrange(
                                "p (w s) -> p w s", w=w))
                    # all heads for this s-range done: ship + gather the slab
                    # every second g. gpsimd is otherwise idle in B, so the
                    # collective's short input wait can't block anything.
                    if g % 2 == 1:
                        gi = g // 2
                        nc.sync.dma_start(
                            cc_ains[gi].rearrange("h p s -> p h s"),
                            attnT[:, :, (g - 1) * 256:(g + 1) * 256])
                        nc.gpsimd.collective_compute(
                            "AllGather", ALU.bypass,
                            replica_groups=C.groups,
                            ins=[cc_ains[gi].opt()], outs=[cc_aouts[gi].opt()])
                    if g == 0 or g == 2:
                        # last slices' rope chains: their AllReduces land
                        # after A ends, and B only touches slice 2 from g=4
                        # and slice 3 from g=6 onward.
                        emit_rchain(C.NST - 2 + g // 2)

        # ---- phase C: output projection ----------------------------------
        # wo resident in three column panels; the small panel and the first
        # lhs sub-slice load first so the PE starts ~12us after B. lhs slabs
        # load whole (1KB-contiguous segments -> cheap descriptors).
        with (
            tc.tile_pool(name="wop", bufs=1) as wop,
            tc.tile_pool(name="lhsp", bufs=2) as lhsp,
            tc.tile_pool(name="outp", bufs=3) as outp,
            tc.tile_pool(name="psO", bufs=3, space="PSUM") as psO,
        ):
            col_ts = [(1024, 256), (0, 512), (512, 512)]
            wo_ps = [wop.tile([128, C.HQ, wdt], BF16, tag=f"wop{ci}",
                              name=f"wop{ci}")
                     for ci, (c0, wdt) in enumerate(col_ts)]
            nc.sync.dma_start(wo_ps[0][:], wo_in[:, :, 1024:1280])

            def c_block(blk, lhs, cis):
                for ci in cis:
                    c0, wdt = col_ts[ci]
                    for sbl in range(4):
                        sb = blk * 4 + sbl
                        off = sbl * 128
                        ps_o = psO.tile([128, 512], F32, tag="o")
                        for slot in range(C.HQ):
                            nc.tensor.matmul(
                                ps_o[:, :wdt],
                                lhs[:, slot, off:off + 128],
                                wo_ps[ci][:, slot, :],
                                start=(slot == 0),
                                stop=(slot == C.HQ - 1))
                        oro = outp.tile([128, 512], F32, tag="oro")
                        nc.vector.tensor_copy(oro[:, :wdt], ps_o[:, :wdt])
                        nc.sync.dma_start(
                            out_sh[sb * 128:(sb + 1) * 128, c0:c0 + wdt],
                            oro[:, :wdt])

            lhs0 = lhsp.tile([128, C.HQ, 512], BF16, tag="lhs")
            nc.sync.dma_start(
                lhs0[:],
                cc_aouts[0].rearrange("r h p s -> p (r h) s"))
            c_block(0, lhs0, [0])
            nc.sync.dma_start(wo_ps[1][:], wo_in[:, :, 0:512])
            nc.sync.dma_start(wo_ps[2][:], wo_in[:, :, 512:1024])
            c_block(0, lhs0, [1, 2])
            for blk in range(1, NGA):
                lhs = lhsp.tile([128, C.HQ, 512], BF16, tag="lhs")
                nc.sync.dma_start(
                    lhs[:],
                    cc_aouts[blk].rearrange("r h p s -> p (r h) s"))
                c_block(blk, lhs, [0, 1, 2])


def build_program(C):
    nc = bacc.Bacc("TRN2", target_bir_lowering=False, debug=False,
                   num_devices=C.NC)
    io = {
        "xT16": nc.dram_tensor("xT16", [C.KC, 128, C.S], BF16, kind="ExternalInput").ap(),
        "wqkv": nc.dram_tensor("wqkv", [C.HCL, 128, C.KC, 128], BF16,
                               kind="ExternalInput").ap(),
        "wv_in": nc.dram_tensor("wv_in", [128, C.KC, C.KVL * 128], BF16,
                                kind="ExternalInput").ap(),
        "wo_in": nc.dram_tensor("wo_in", [128, C.HQ, C.COLS], BF16,
                                kind="ExternalInput").ap(),
        "tabqc": nc.dram_tensor("tabqc", [128, C.S], BF16, kind="ExternalInput").ap(),
        "tabqs": nc.dram_tensor("tabqs", [128, C.S], BF16, kind="ExternalInput").ap(),
        "tabkc": nc.dram_tensor("tabkc", [128, C.S], BF16, kind="ExternalInput").ap(),
        "tabks": nc.dram_tensor("tabks", [128, C.S], BF16, kind="ExternalInput").ap(),
        "masks": nc.dram_tensor("masks", [128, 4, 256], F32, kind="ExternalInput").ap(),
        "wnorm": nc.dram_tensor("wnorm", [128, C.HQL + C.KVL], F32,
                                kind="ExternalInput").ap(),
        "out_sh": nc.dram_tensor("out_sh", [C.S, C.COLS], F32,
                                 kind="ExternalOutput").ap(),
    }
    with tile.TileContext(nc) as tc:
        attention_tile_kernel(tc, C, io)
    nc.compile()
    return nc


def make_masks(mask_np, C):
    """4 mask tiles [t,s-pair] for offsets {0,1,WD,WD+1}; returns [128,4,256] f32."""
    S, WD, SW = C.S, C.WD, C.SW
    I0 = WD + 1

    def tileT(d):
        i, j = I0, I0 - d
        if 0 <= j < C.NT:
            blk = np.array(mask_np[i * 128:(i + 1) * 128, j * 128:(j + 1) * 128],
                           dtype=np.float64)
        else:
            blk = np.full((128, 128), -np.inf)
        s_idx = np.arange(128)[:, None]
        t_idx = np.arange(128)[None, :]
        dist = 128 * d + s_idx - t_idx
        blk = np.where(dist > SW, -np.inf, blk)
        return np.maximum(blk.T, -1e30).astype(np.float32)   # [t, s]

    tiles = []
    for off in (0, 1, WD, WD + 1):
        dl, dr = off - 1, off
        tiles.append(np.concatenate([tileT(dl), tileT(dr)], axis=1))
    return np.ascontiguousarray(np.stack(tiles, axis=1))      # [128, 4, 256]


def make_core_inputs(inputs, C):
    x = np.asarray(inputs["x"], dtype=np.float32)
    wq = np.asarray(inputs["wq"], dtype=np.float32)
    wk = np.asarray(inputs["wk"], dtype=np.float32)
    wv = np.asarray(inputs["wv"], dtype=np.float32)
    wo = np.asarray(inputs["wo"], dtype=np.float32)
    qw = np.asarray(inputs["q_norm_weight"], dtype=np.float32)
    kw = np.asarray(inputs["k_norm_weight"], dtype=np.float32)
    ch = np.asarray(inputs["cos_half"], dtype=np.float32)
    sh = np.asarray(inputs["sin_half"], dtype=np.float32)
    mask = np.asarray(inputs["mask"], dtype=np.float32)
    assert int(inputs.get("start_pos", 0) or 0) == 0

    cosT = np.ascontiguousarray(np.concatenate([ch.T, ch.T], axis=0))
    sinT = np.ascontiguousarray(np.concatenate([-sh.T, sh.T], axis=0))
    # norm constants folded into the tables: r = raw^-0.5 on device
    cstq = C.CQ * np.sqrt(C.DIM)
    cstk = np.sqrt(C.HKV * 128.0)
    tabqc = (cosT * cstq).astype(BF16_NP)
    tabqs = (sinT * cstq).astype(BF16_NP)
    tabkc = (cosT * cstk).astype(BF16_NP)
    tabks = (sinT * cstk).astype(BF16_NP)
    masks = make_masks(mask, C)
    KC, HQL, KVL = C.KC, C.HQL, C.KVL

    xT_cache = {}
    for b in range(C.DP):
        xT_cache[b] = np.ascontiguousarray(x[b].T).astype(BF16_NP).reshape(
            C.KC, 128, C.S)
    in_maps = []
    for c in range(C.NC):
        b, q4 = c // C.TP, c % C.TP
        x16 = xT_cache[b]
        wq_s = wq[:, 128 * HQL * q4:128 * HQL * (q4 + 1)]
        wk_s = wk[:, 128 * KVL * q4:128 * KVL * (q4 + 1)]
        wv_s = wv[:, 128 * KVL * q4:128 * KVL * (q4 + 1)]
        wqk = np.concatenate([wq_s, wk_s], axis=1).astype(BF16_NP)
        # [HCL, 128, KC, 128]: per chain, contraction-partition-major
        wqkv_pre = np.ascontiguousarray(
            wqk.reshape(KC, 128, C.HCL, 128).transpose(2, 1, 0, 3))
        wv_pre = np.ascontiguousarray(
            wv_s.astype(BF16_NP).reshape(KC, 128, KVL * 128).transpose(1, 0, 2))
        wo_s = wo[:, C.COLS * q4:C.COLS * (q4 + 1)].astype(BF16_NP)
        wo_pre = np.ascontiguousarray(
            wo_s.reshape(C.HQ, 128, C.COLS).transpose(1, 0, 2))
        wn = np.zeros((128, HQL + KVL), dtype=np.float32)
        for hc in range(HQL):
            g = HQL * q4 + hc
            wn[:, hc] = qw[128 * g:128 * (g + 1)]
        for j in range(KVL):
            g = KVL * q4 + j
            wn[:, HQL + j] = kw[128 * g:128 * (g + 1)]
        in_maps.append({"xT16": x16, "wqkv": wqkv_pre, "wv_in": wv_pre,
                        "wo_in": wo_pre, "tabqc": tabqc, "tabqs": tabqs,
                        "tabkc": tabkc, "tabks": tabks,
                        "masks": masks, "wnorm": wn})
    return in_maps


_CACHED = {}


def run(inputs, C=None, trace=False, stitch=None, trace_cores=None):
    C = C or Cfg()
    key = (C.S, C.DIM, C.HQ, C.HKV, C.TP, C.DP, C.SW)
    if key not in _CACHED:
        _CACHED[key] = build_program(C)
    nc = _CACHED[key]
    in_maps = make_core_inputs(inputs, C)
    if stitch is None:
        stitch = trace
    if trace and trace_cores is None:
        trace_cores = list(range(C.NC))
    res = bass_utils.run_bass_kernel_spmd(
        nc, in_maps, core_ids=list(range(C.NC)), trace=trace,
        stitch_traces=stitch, trace_cores=trace_cores if trace else None)
    out = np.empty((C.DP, C.S, C.DIM), dtype=np.float32)
    for c in range(C.NC):
        b, q4 = c // C.TP, c % C.TP
        out[b, :, C.COLS * q4:C.COLS * (q4 + 1)] = res.results[c]["out_sh"]
    return out, res


def kernel(**inputs) -> np.ndarray:
    out, _ = run(inputs)
    return out


# revision 53
# speedup vs baseline: 1.1495x; 1.0474x over previous
"""Sparse (sliding-window) GQA attention prefill kernel for 8 Trainium2 cores.

Sharding: TP=4 over KV heads (2 KV heads + 10 Q heads per core) x DP=2 over
batch. Core c: batch = c // 4, shard q4 = c % 4.

Device program (SPMD, identical on all cores; per-core data via inputs):
  A1: xqT/xkT projections in transposed layout ([head_dim, seq]); sum-of-squares
      for the global RMS norm via Square + ones-matmul; per-s-tile AllReduce of
      the norm partials within each batch group (overlaps A1 compute).
  A2: V projection in natural layout ([seq, head_dim]).
  R:  norm rows -> rope tables (bf16, norm factor folded in); batched rope over
      all local heads per 512-slice (vector for q, gpsimd for k), overlapping A2.
  B:  per (head-pair chunk, 2-query-tile group): scoresT = K^T-chunk.T @ qT in
      the sliding band, mask add, exp, ones-matmul denominator, P^T @ V
      accumulation. Software-pipelined: the denominator/PV matmuls for step j
      are emitted after the scores matmul of step j+1 so the PE never waits on
      the exp. Divide on evacuation via broadcast + reciprocal on [128, .].
      attnT shipped in s-halves: AllGather per (chunk, half), issued late so the
      collective's input wait never blocks the gpsimd queue.
  C:  out = attnT.T @ wo col-shard; first s-half rows start as soon as the
      half-0 gathers land.
"""

import sys
import numpy as np

for _p in ("/opt/trn_rl_repo", "/root/.axon_site/_ro/trn_rl_repo"):
    if _p not in sys.path:
        sys.path.insert(0, _p)

import ml_dtypes

import concourse.bass as bass
import concourse.tile as tile
from concourse import bacc, mybir
from concourse import bass_utils

F32 = mybir.dt.float32
BF16 = mybir.dt.bfloat16
BF16_NP = ml_dtypes.bfloat16
AF = mybir.ActivationFunctionType
ALU = mybir.AluOpType


class Cfg:
    def __init__(self, S=2048, DIM=5120, HQ=40, HKV=8, TP=4, DP=2, SW=1024,
                 MSCALE=1.2079441541679836, EPS=1e-6):
        self.S, self.DIM, self.HQ, self.HKV = S, DIM, HQ, HKV
        self.TP, self.DP, self.SW = TP, DP, SW
        self.MSCALE, self.EPS = MSCALE, EPS
        self.D = 128
        self.NC = TP * DP
        self.HQL = HQ // TP          # local q heads
        self.KVL = HKV // TP         # local kv heads
        self.REP = HQ // HKV
        self.KC = DIM // 128         # contraction chunks
        self.NT = S // 128           # seq tiles
        self.G = self.NT // 2        # 2-query-tile groups
        self.WD = SW // 128          # window in tiles
        self.COLS = DIM // TP        # output column shard
        self.HCL = self.HQL + self.KVL  # projection chains with transposed out
        self.NST = S // 512          # 512-wide s-tiles (phase A1)
        self.NST2 = S // 256         # 256-wide s-tiles (phase A2)
        self.CQ = self.D ** -0.5 * MSCALE
        assert self.WD >= 2 and self.NT > self.WD + 1 and self.NT % 2 == 0
        self.groups = [[b * TP + r for r in range(TP)] for b in range(DP)]


def head_chunks(C):
    """Per-kv head pair chunks: [(kv, [h0,h1]), (kv, [h2,h3]), (kv, [h4])...]"""
    out = []
    per = C.HQL // C.KVL
    for kv in range(C.KVL):
        hs = list(range(kv * per, (kv + 1) * per))
        i = 0
        while i < len(hs):
            out.append((kv, hs[i:i + 2]))
            i += 2
    return out


def attention_tile_kernel(tc, C, io):
    nc = tc.nc
    S, KC, HQL, KVL, NT, G, WD = C.S, C.KC, C.HQL, C.KVL, C.NT, C.G, C.WD
    H2 = S // 2
    xT16, wqkv, wv_in, wo_in = io["xT16"], io["wqkv"], io["wv_in"], io["wo_in"]
    tabqc_in, tabqs_in = io["tabqc"], io["tabqs"]
    tabkc_in, tabks_in = io["tabkc"], io["tabks"]
    masks_in, wnorm_in = io["masks"], io["wnorm"]
    out_sh = io["out_sh"]
    chunks = head_chunks(C)

    from contextlib import ExitStack
    ctx = ExitStack()
    with ctx:
        singles = ctx.enter_context(tc.tile_pool(name="singles", bufs=1))
        dramcc = ctx.enter_context(tc.tile_pool(name="dramcc", bufs=1, space="DRAM"))

        ones16 = singles.tile([128, 1], BF16)
        nc.vector.memset(ones16[:], 1.0)
        ones128 = singles.tile([128, 128], BF16)
        nc.vector.memset(ones128[:], 1.0)
        wnorm_sb = singles.tile([128, HQL + KVL], F32)
        nc.sync.dma_start(wnorm_sb[:], wnorm_in[:])

        cc_nins = [dramcc.tile([1, 1024], F32, name=f"ccni{st}")
                   for st in range(C.NST)]
        cc_nouts = [dramcc.tile([1, 1024], F32, name=f"ccno{st}")
                    for st in range(C.NST)]
        # one gather per 512-wide s-range (phase B runs g-outer), so phase C
        # consumes s-slabs progressively and never waits on the last
        # collective; 4 gathers keeps the ~40us fixed CC cost per collective
        # well under phase B's span
        NGA = G // 2
        cc_ains = [dramcc.tile([HQL, 128, 512], BF16, name=f"ccag{gi}")
                   for gi in range(NGA)]
        cc_aouts = [dramcc.tile([C.TP, HQL, 128, 512], BF16,
                                name=f"ccaog{gi}") for gi in range(NGA)]

        wvp = ctx.enter_context(tc.tile_pool(name="wvp", bufs=1))
        wv_sb = wvp.tile([128, KC, KVL * 128], BF16)
        nc.gpsimd.dma_start(wv_sb[:], wv_in[:])

        with (
            tc.tile_pool(name="xqp", bufs=1) as xq_pool,
            tc.tile_pool(name="xkp", bufs=1) as xk_pool,
            tc.tile_pool(name="vp", bufs=1) as v_pool,
            tc.tile_pool(name="tabsQ", bufs=1) as tabs_q,
            tc.tile_pool(name="rowsp", bufs=1) as rowsp,
            tc.tile_pool(name="ropep", bufs=1) as ropep,
        ):
            xq_sb = xq_pool.tile([128, HQL, S], BF16)
            xk_sb = xk_pool.tile([128, KVL, S], BF16)
            v_sb = v_pool.tile([128, NT, KVL, 128], BF16)

            # ---- phase A: q/k projections (transposed out) + norm
            # partials + V projection, all per 512-wide s-tile. V reuses the
            # same xt tiles as stationary, so x is loaded exactly once.
            # Rope chains are emitted with a 2-tile lag so each slice's
            # AllReduce has completed long before its chain runs; slices 0-1
            # rope on the vector engine during A itself.
            KCH = KC // 2
            with (
                tc.tile_pool(name="xt1", bufs=2) as xt1,
                tc.tile_pool(name="wst", bufs=2) as wst,
                tc.tile_pool(name="sqp", bufs=2) as sqp,
                tc.tile_pool(name="trow", bufs=1) as trow,
                tc.tile_pool(name="psA", bufs=3, space="PSUM") as psA,
                tc.tile_pool(name="psN", bufs=1, space="PSUM") as psN,
                tc.tile_pool(name="psV", bufs=3, space="PSUM") as psV,
            ):

                def emit_rchain(st):
                    sl = slice(st * 512, (st + 1) * 512)
                    # r = raw^-0.5 for both rows in one Ln/Exp pass (norm
                    # constants are folded into the host-prescaled tables)
                    rowraw = rowsp.tile([1, 1024], F32, tag="rowraw")
                    nc.gpsimd.dma_start(rowraw[:], cc_nouts[st][:])
                    rb = rowsp.tile([128, 1024], F32, tag="rb")
                    nc.gpsimd.partition_broadcast(rb[:], rowraw[:])
                    nc.scalar.activation(rb[:], rb[:], AF.Ln)
                    nc.scalar.activation(rb[:], rb[:], AF.Exp, scale=-0.5)
                    cosq = tabs_q.tile([128, 512], BF16, tag="cosq")
                    sinq = tabs_q.tile([128, 512], BF16, tag="sinq")
                    cosk = tabs_q.tile([128, 512], BF16, tag="cosk")
                    sink = tabs_q.tile([128, 512], BF16, tag="sink")
                    nc.gpsimd.dma_start(cosk[:], tabkc_in[:, sl])
                    nc.gpsimd.dma_start(sink[:], tabks_in[:, sl])
                    nc.gpsimd.dma_start(cosq[:], tabqc_in[:, sl])
                    nc.gpsimd.dma_start(sinq[:], tabqs_in[:, sl])
                    nc.vector.tensor_mul(cosk[:], cosk[:], rb[:, 512:1024])
                    nc.vector.tensor_mul(sink[:], sink[:], rb[:, 512:1024])
                    nc.vector.tensor_mul(cosq[:], cosq[:], rb[:, 0:512])
                    nc.vector.tensor_mul(sinq[:], sinq[:], rb[:, 0:512])
                    rotk = ropep.tile([128, KVL, 512], BF16, tag="rotk")
                    rotq = ropep.tile([128, HQL, 512], BF16, tag="rotq")
                    nc.gpsimd.dma_start(rotk[0:64], xk_sb[64:128, :, sl])
                    nc.gpsimd.dma_start(rotk[64:128], xk_sb[0:64, :, sl])
                    nc.gpsimd.dma_start(rotq[0:64], xq_sb[64:128, :, sl])
                    nc.gpsimd.dma_start(rotq[64:128], xq_sb[0:64, :, sl])
                    # fully in-place on vector: rot *= sin; x *= cos; x += rot
                    nc.vector.tensor_mul(
                        rotk[:], rotk[:],
                        sink[:, None, :].to_broadcast((128, KVL, 512)))
                    nc.vector.tensor_mul(
                        xk_sb[:, :, sl], xk_sb[:, :, sl],
                        cosk[:, None, :].to_broadcast((128, KVL, 512)))
                    nc.vector.tensor_add(xk_sb[:, :, sl], xk_sb[:, :, sl],
                                         rotk[:])
                    nc.vector.tensor_mul(
                        rotq[:], rotq[:],
                        sinq[:, None, :].to_broadcast((128, HQL, 512)))
                    nc.vector.tensor_mul(
                        xq_sb[:, :, sl], xq_sb[:, :, sl],
                        cosq[:, None, :].to_broadcast((128, HQL, 512)))
                    nc.vector.tensor_add(xq_sb[:, :, sl], xq_sb[:, :, sl],
                                         rotq[:])

                for st in range(C.NST):
                    s0 = st * 512
                    xt_a = xt1.tile([128, KCH, 512], BF16, tag="xta")
                    xt_b = xt1.tile([128, KCH, 512], BF16, tag="xtb")
                    nc.sync.dma_start(
                        xt_a[:],
                        xT16[:KCH, :, s0:s0 + 512].rearrange("kc p s -> p kc s"))
                    nc.sync.dma_start(
                        xt_b[:],
                        xT16[KCH:, :, s0:s0 + 512].rearrange("kc p s -> p kc s"))
                    ps_nq = psN.tile([1, 512], F32, tag="nq")
                    ps_nk = psN.tile([1, 512], F32, tag="nk")
                    for hc in range(C.HCL):
                        if hc == 6 and st >= 2:
                            emit_rchain(st - 2)
                        w_sb = wst.tile([128, KC, 128], BF16, tag="w")
                        nc.sync.dma_start(w_sb[:], wqkv[hc])
                        ps = psA.tile([128, 512], F32, tag="proj")
                        for kc in range(KC):
                            xsrc = xt_a if kc < KCH else xt_b
                            nc.tensor.matmul(ps[:], w_sb[:, kc, :],
                                             xsrc[:, kc % KCH, :],
                                             start=(kc == 0), stop=(kc == KC - 1))
                        if hc < HQL:
                            dest = xq_sb[:, hc, s0:s0 + 512]
                        else:
                            dest = xk_sb[:, hc - HQL, s0:s0 + 512]
                        nc.vector.tensor_scalar_mul(dest, ps[:],
                                                    wnorm_sb[:, hc:hc + 1])
                        sq = sqp.tile([128, 512], BF16, tag="sq")
                        nc.scalar.activation(sq[:], ps[:], AF.Square)
                        tgt = ps_nq if hc < HQL else ps_nk
                        first = (hc == 0) or (hc == HQL)
                        last = (hc == HQL - 1) or (hc == C.HCL - 1)
                        nc.tensor.matmul(tgt[:], ones16[:], sq[:],
                                         start=first, stop=last)
                    rq_t = trow.tile([1, 512], F32, tag="rq")
                    rk_t = trow.tile([1, 512], F32, tag="rk")
                    nc.vector.tensor_copy(rq_t[:], ps_nq[:])
                    nc.vector.tensor_copy(rk_t[:], ps_nk[:])
                    nc.sync.dma_start(cc_nins[st][0:1, 0:512], rq_t[:])
                    nc.sync.dma_start(cc_nins[st][0:1, 512:1024], rk_t[:])
                    # per-s-tile AllReduce of norm partials: overlaps A compute
                    nc.gpsimd.collective_compute(
                        "AllReduce", ALU.add, replica_groups=C.groups,
                        ins=[cc_nins[st].opt()], outs=[cc_nouts[st].opt()])
                    # V projection for this s-tile, x chunks as stationary
                    for tc4 in range(4):
                        tt = st * 4 + tc4
                        psv = psV.tile([128, KVL * 128], F32, tag="v")
                        for kc in range(KC):
                            xsrc = xt_a if kc < KCH else xt_b
                            nc.tensor.matmul(
                                psv[:],
                                xsrc[:, kc % KCH,
                                     tc4 * 128:(tc4 + 1) * 128],
                                wv_sb[:, kc, :],
                                start=(kc == 0), stop=(kc == KC - 1))
                        nc.scalar.copy(v_sb[:, tt, :, :], psv[:])


            # ---- phase B: banded attention, software-pipelined --------------
            with (
                tc.tile_pool(name="attnp", bufs=1) as attnp,
                tc.tile_pool(name="maskp", bufs=1) as maskp,
                tc.tile_pool(name="expp", bufs=4) as expp,
                tc.tile_pool(name="bmisc", bufs=3) as bmisc,
                tc.tile_pool(name="psSC", bufs=2, space="PSUM") as psSC,
                tc.tile_pool(name="psAT", bufs=2, space="PSUM") as psAT,
                tc.tile_pool(name="psDN", bufs=2, space="PSUM") as psDN,
            ):
                attnT = attnp.tile([128, HQL, S], BF16)
                masks_sb = maskp.tile([128, 4, 256], F32)
                nc.sync.dma_start(masks_sb[:], masks_in[:])
                off2m = {0: 0, 1: 1, WD: 2, WD + 1: 3}

                pend = []  # (ex slice, j, group-state) awaiting dn/at

                def evac(stt):
                    w, h0, g = stt["w"], stt["h0"], stt["g"]
                    den_b = bmisc.tile([128, 512], F32, tag="denb")
                    nc.vector.reciprocal_approx_fast(
                        out=den_b[:, :w * 256], in_=stt["dn"][:, :w * 256])
                    nc.vector.tensor_mul(
                        attnT[:, h0:h0 + w, g * 256:(g + 1) * 256],
                        stt["at"][:, :w * 256].rearrange(
                            "p (w s) -> p w s", w=w),
                        den_b[:, :w * 256].rearrange(
                            "p (w s) -> p w s", w=w))

                def drain_one():
                    ex_ap, j, stt = pend.pop(0)
                    w, kv = stt["w"], stt["kv"]
                    nc.tensor.matmul(stt["dn"][:, :w * 256], ones128[:],
                                     ex_ap, start=(j == stt["jlo"]),
                                     stop=(j == stt["jhi"]))
                    nc.tensor.matmul(stt["at"][:, :w * 256],
                                     v_sb[:, j, kv, :],
                                     ex_ap, start=(j == stt["jlo"]),
                                     stop=(j == stt["jhi"]))
                    if j == stt["jhi"]:
                        evac(stt)

                for g in range(G):
                    jlo, jhi = max(0, 2 * g - WD), 2 * g + 1
                    npairs = (jhi - jlo + 1) // 2
                    for ci, (kv, hs) in enumerate(chunks):
                        w = len(hs)
                        h0 = hs[0]
                        stt = {
                            "w": w, "kv": kv, "h0": h0, "g": g,
                            "jlo": jlo, "jhi": jhi,
                            # ones128 stationary -> denominator lands
                            # pre-broadcast across all partitions
                            "at": psAT.tile([128, 512], F32, tag="at",
                                            name="ps_at"),
                            "dn": psDN.tile([128, 512], F32, tag="dn",
                                            name="ps_dn"),
                        }
                        for p in range(npairs):
                            j0 = jlo + 2 * p
                            ps2 = psSC.tile([128, 1024], F32, tag="sc")
                            for dj in range(2):
                                j = j0 + dj
                                o = dj * 512
                                nc.tensor.matmul(
                                    ps2[:, o:o + w * 256].rearrange(
                                        "p (w s) -> p w s", w=w),
                                    xk_sb[:, kv, j * 128:(j + 1) * 128],
                                    xq_sb[:, h0:h0 + w,
                                          g * 256:(g + 1) * 256],
                                    start=True, stop=True)
                            # consume older pairs (also across group
                            # boundaries) so the drain's exp latency is
                            # never exposed at a boundary
                            while len(pend) > 2:
                                drain_one()
                            for dj in range(2):
                                j = j0 + dj
                                m = off2m.get(jhi - j)
                                if m is not None:
                                    o = dj * 512
                                    nc.vector.tensor_add(
                                        ps2[:, o:o + w * 256].rearrange(
                                            "p (w s) -> p w s", w=w),
                                        ps2[:, o:o + w * 256].rearrange(
                                            "p (w s) -> p w s", w=w),
                                        masks_sb[:, m, None,
                                                 :].to_broadcast(
                                            (128, w, 256)))
                            ex2 = expp.tile([128, 1024], BF16, tag="ex")
                            nc.scalar.activation(
                                ex2.rearrange("p (j s) -> p j s",
                                              j=2)[:, :, :w * 256],
                                ps2.rearrange("p (j s) -> p j s",
                                              j=2)[:, :, :w * 256],
                                AF.Exp)
                            pend.append((ex2[:, 0:w * 256], j0, stt))
                            pend.append((ex2[:, 512:512 + w * 256],
                                         j0 + 1, stt))
                    if g % 2 == 1:
                        # drain so the slab DMA is emitted after the evacs
                        # it reads, then ship + gather
                        while pend:
                            drain_one()
                        gi = g // 2
                        nc.sync.dma_start(
                            cc_ains[gi].rearrange("h p s -> p h s"),
                            attnT[:, :, (g - 1) * 256:(g + 1) * 256])
                        nc.gpsimd.collective_compute(
                            "AllGather", ALU.bypass,
                            replica_groups=C.groups,
                            ins=[cc_ains[gi].opt()],
                            outs=[cc_aouts[gi].opt()])
                    if g == 0 or g == 2:
                        # last slices' rope chains: their AllReduces land
                        # after A ends, and B only touches slice 2 from g=4
                        # and slice 3 from g=6 onward.
                        emit_rchain(C.NST - 2 + g // 2)
                while pend:
                    drain_one()

        # ---- phase C: output projection ----------------------------------
        # wo resident in three column panels; the small panel and the first
        # lhs sub-slice load first so the PE starts ~12us after B. lhs slabs
        # load whole (1KB-contiguous segments -> cheap descriptors).
        with (
            tc.tile_pool(name="wop", bufs=1) as wop,
            tc.tile_pool(name="lhsp", bufs=2) as lhsp,
            tc.tile_pool(name="outp", bufs=3) as outp,
            tc.tile_pool(name="psO", bufs=3, space="PSUM") as psO,
        ):
            col_ts = [(1024, 256), (0, 512), (512, 512)]
            wo_ps = [wop.tile([128, C.HQ, wdt], BF16, tag=f"wop{ci}",
                              name=f"wop{ci}")
                     for ci, (c0, wdt) in enumerate(col_ts)]
            nc.sync.dma_start(wo_ps[0][:], wo_in[:, :, 1024:1280])

            def c_block(blk, lhs, cis):
                for ci in cis:
                    c0, wdt = col_ts[ci]
                    for sbl in range(4):
                        sb = blk * 4 + sbl
                        off = sbl * 128
                        ps_o = psO.tile([128, 512], F32, tag="o")
                        for slot in range(C.HQ):
                            nc.tensor.matmul(
                                ps_o[:, :wdt],
                                lhs[:, slot, off:off + 128],
                                wo_ps[ci][:, slot, :],
                                start=(slot == 0),
                                stop=(slot == C.HQ - 1))
                        oro = outp.tile([128, 512], F32, tag="oro")
                        nc.vector.tensor_copy(oro[:, :wdt], ps_o[:, :wdt])
                        nc.sync.dma_start(
                            out_sh[sb * 128:(sb + 1) * 128, c0:c0 + wdt],
                            oro[:, :wdt])

            lhs0 = lhsp.tile([128, C.HQ, 512], BF16, tag="lhs")
            nc.sync.dma_start(
                lhs0[:],
                cc_aouts[0].rearrange("r h p s -> p (r h) s"))
            c_block(0, lhs0, [0])
            nc.sync.dma_start(wo_ps[1][:], wo_in[:, :, 0:512])
            nc.sync.dma_start(wo_ps[2][:], wo_in[:, :, 512:1024])
            c_block(0, lhs0, [1, 2])
            for blk in range(1, NGA):
                lhs = lhsp.tile([128, C.HQ, 512], BF16, tag="lhs")
                nc.sync.dma_start(
                    lhs[:],
                    cc_aouts[blk].rearrange("r h p s -> p (r h) s"))
                c_block(blk, lhs, [0, 1, 2])


def build_program(C):
    nc = bacc.Bacc("TRN2", target_bir_lowering=False, debug=False,
                   num_devices=C.NC)
    io = {
        "xT16": nc.dram_tensor("xT16", [C.KC, 128, C.S], BF16, kind="ExternalInput").ap(),
        "wqkv": nc.dram_tensor("wqkv", [C.HCL, 128, C.KC, 128], BF16,
                               kind="ExternalInput").ap(),
        "wv_in": nc.dram_tensor("wv_in", [128, C.KC, C.KVL * 128], BF16,
                                kind="ExternalInput").ap(),
        "wo_in": nc.dram_tensor("wo_in", [128, C.HQ, C.COLS], BF16,
                                kind="ExternalInput").ap(),
        "tabqc": nc.dram_tensor("tabqc", [128, C.S], BF16, kind="ExternalInput").ap(),
        "tabqs": nc.dram_tensor("tabqs", [128, C.S], BF16, kind="ExternalInput").ap(),
        "tabkc": nc.dram_tensor("tabkc", [128, C.S], BF16, kind="ExternalInput").ap(),
        "tabks": nc.dram_tensor("tabks", [128, C.S], BF16, kind="ExternalInput").ap(),
        "masks": nc.dram_tensor("masks", [128, 4, 256], F32, kind="ExternalInput").ap(),
        "wnorm": nc.dram_tensor("wnorm", [128, C.HQL + C.KVL], F32,
                                kind="ExternalInput").ap(),
        "out_sh": nc.dram_tensor("out_sh", [C.S, C.COLS], F32,
                                 kind="ExternalOutput").ap(),
    }
    with tile.TileContext(nc) as tc:
        attention_tile_kernel(tc, C, io)
    nc.compile()
    return nc


def make_masks(mask_np, C):
    """4 mask tiles [t,s-pair] for offsets {0,1,WD,WD+1}; returns [128,4,256] f32."""
    S, WD, SW = C.S, C.WD, C.SW
    I0 = WD + 1

    def tileT(d):
        i, j = I0, I0 - d
        if 0 <= j < C.NT:
            blk = np.array(mask_np[i * 128:(i + 1) * 128, j * 128:(j + 1) * 128],
                           dtype=np.float64)
        else:
            blk = np.full((128, 128), -np.inf)
        s_idx = np.arange(128)[:, None]
        t_idx = np.arange(128)[None, :]
        dist = 128 * d + s_idx - t_idx
        blk = np.where(dist > SW, -np.inf, blk)
        return np.maximum(blk.T, -1e30).astype(np.float32)   # [t, s]

    tiles = []
    for off in (0, 1, WD, WD + 1):
        dl, dr = off - 1, off
        tiles.append(np.concatenate([tileT(dl), tileT(dr)], axis=1))
    return np.ascontiguousarray(np.stack(tiles, axis=1))      # [128, 4, 256]


def make_core_inputs(inputs, C):
    x = np.asarray(inputs["x"], dtype=np.float32)
    wq = np.asarray(inputs["wq"], dtype=np.float32)
    wk = np.asarray(inputs["wk"], dtype=np.float32)
    wv = np.asarray(inputs["wv"], dtype=np.float32)
    wo = np.asarray(inputs["wo"], dtype=np.float32)
    qw = np.asarray(inputs["q_norm_weight"], dtype=np.float32)
    kw = np.asarray(inputs["k_norm_weight"], dtype=np.float32)
    ch = np.asarray(inputs["cos_half"], dtype=np.float32)
    sh = np.asarray(inputs["sin_half"], dtype=np.float32)
    mask = np.asarray(inputs["mask"], dtype=np.float32)
    assert int(inputs.get("start_pos", 0) or 0) == 0

    cosT = np.ascontiguousarray(np.concatenate([ch.T, ch.T], axis=0))
    sinT = np.ascontiguousarray(np.concatenate([-sh.T, sh.T], axis=0))
    # norm constants folded into the tables: r = raw^-0.5 on device
    cstq = C.CQ * np.sqrt(C.DIM)
    cstk = np.sqrt(C.HKV * 128.0)
    tabqc = (cosT * cstq).astype(BF16_NP)
    tabqs = (sinT * cstq).astype(BF16_NP)
    tabkc = (cosT * cstk).astype(BF16_NP)
    tabks = (sinT * cstk).astype(BF16_NP)
    masks = make_masks(mask, C)
    KC, HQL, KVL = C.KC, C.HQL, C.KVL

    xT_cache = {}
    for b in range(C.DP):
        xT_cache[b] = np.ascontiguousarray(x[b].T).astype(BF16_NP).reshape(
            C.KC, 128, C.S)
    in_maps = []
    for c in range(C.NC):
        b, q4 = c // C.TP, c % C.TP
        x16 = xT_cache[b]
        wq_s = wq[:, 128 * HQL * q4:128 * HQL * (q4 + 1)]
        wk_s = wk[:, 128 * KVL * q4:128 * KVL * (q4 + 1)]
        wv_s = wv[:, 128 * KVL * q4:128 * KVL * (q4 + 1)]
        wqk = np.concatenate([wq_s, wk_s], axis=1).astype(BF16_NP)
        # [HCL, 128, KC, 128]: per chain, contraction-partition-major
        wqkv_pre = np.ascontiguousarray(
            wqk.reshape(KC, 128, C.HCL, 128).transpose(2, 1, 0, 3))
        wv_pre = np.ascontiguousarray(
            wv_s.astype(BF16_NP).reshape(KC, 128, KVL * 128).transpose(1, 0, 2))
        wo_s = wo[:, C.COLS * q4:C.COLS * (q4 + 1)].astype(BF16_NP)
        wo_pre = np.ascontiguousarray(
            wo_s.reshape(C.HQ, 128, C.COLS).transpose(1, 0, 2))
        wn = np.zeros((128, HQL + KVL), dtype=np.float32)
        for hc in range(HQL):
            g = HQL * q4 + hc
            wn[:, hc] = qw[128 * g:128 * (g + 1)]
        for j in range(KVL):
            g = KVL * q4 + j
            wn[:, HQL + j] = kw[128 * g:128 * (g + 1)]
        in_maps.append({"xT16": x16, "wqkv": wqkv_pre, "wv_in": wv_pre,
                        "wo_in": wo_pre, "tabqc": tabqc, "tabqs": tabqs,
                        "tabkc": tabkc, "tabks": tabks,
                        "masks": masks, "wnorm": wn})
    return in_maps


_CACHED = {}


def run(inputs, C=None, trace=False, stitch=None, trace_cores=None):
    C = C or Cfg()
    key = (C.S, C.DIM, C.HQ, C.HKV, C.TP, C.DP, C.SW)
    if key not in _CACHED:
        _CACHED[key] = build_program(C)
    nc = _CACHED[key]
    in_maps = make_core_inputs(inputs, C)
    if stitch is None:
        stitch = trace
    if trace and trace_cores is None:
        trace_cores = list(range(C.NC))
    res = bass_utils.run_bass_kernel_spmd(
        nc, in_maps, core_ids=list(range(C.NC)), trace=trace,
        stitch_traces=stitch, trace_cores=trace_cores if trace else None)
    out = np.empty((C.DP, C.S, C.DIM), dtype=np.float32)
    for c in range(C.NC):
        b, q4 = c // C.TP, c % C.TP
        out[b, :, C.COLS * q4:C.COLS * (q4 + 1)] = res.results[c]["out_sh"]
    return out, res


def kernel(**inputs) -> np.ndarray:
    out, _ = run(inputs)
    return out
